# revision 47
# baseline (speedup 1.0000x reference)
"""Trainium2 Bass kernel for nn_FFTPermeabilityPredictorPatchPhysics.

Sharding: pure data parallel — 8 samples per NeuronCore, weights replicated.
On-device layout: residual stream transposed, hT [3x128 d-chunks, 1576 tok],
kept in SBUF for all 12 layers. FFT/iFFT as block-diagonal matmuls over a
512-row padded frequency layout (head h -> rows 64h+32s+f). LN stats via
ones-matmul partition reductions broadcast to all partitions; the adaptive
spectral filter is fused into the ACT-engine gelu via per-partition
scale/bias. The MLP runs fp8e4 DoubleRow (K=256 per instruction) for the
196 patch tokens with weight scale 64 folded into the gelu scale and the
residual scalar_tensor_tensor; the cls token column (which feeds the head
directly, without the 1/197 mean dilution of patch tokens) is recomputed
in bf16 against the same-layer bf16 weights. All weight folding done
host-side in numpy: double-LN collapse, pre_g/ln2_g into following
matmuls, base_filter and (1+ap) into amlp_w2, 1/197 token-mean into
amlp_w1, DFT matrices baked. Final LN + head on the 64 cls vectors runs
host-side in float64.
"""
import numpy as np

import concourse.bacc as bacc
import concourse.mybir as mybir
import concourse.tile as tile
from concourse.bass_utils import run_bass_kernel_spmd

F32 = mybir.dt.float32
F32R = mybir.dt.float32r
BF16 = mybir.dt.bfloat16
FP8 = mybir.dt.float8e4
AF = mybir.ActivationFunctionType
ALU = mybir.AluOpType
DR = mybir.MatmulPerfMode.DoubleRow

B, D, H, HD, FB, S, L, P, NP_ = 64, 384, 8, 48, 25, 197, 12, 16, 196
EPS = 1e-5
FR = 512
NCORES = 8
BC = B // NCORES     # 8 samples/core
NTOK = BC * S        # 1576
TT = 394             # token tile = 2 samples
NBP = BC * NP_       # 1568
BT = 392             # patch tile = 2 samples
WS = 64.0            # fp8 weight scale for both MLP matmuls
IWS = 1.0 / WS

_CACHE = {}


def _build_dft():
    n = np.arange(HD)
    k = np.arange(FB)
    ang = -2 * np.pi * np.outer(n, k) / HD
    Cr = np.cos(ang) / np.sqrt(HD)
    Ci = np.sin(ang) / np.sqrt(HD)
    A = np.zeros((FB, HD))
    Bm = np.zeros((FB, HD))
    ifft_w = np.exp(2j * np.pi * np.outer(np.arange(HD), np.arange(HD)) / HD) / np.sqrt(HD)
    for j in range(FB):
        fr = np.zeros(HD, complex)
        fi = np.zeros(HD, complex)
        fr[j] = 1.0
        fi[j] = 1.0j
        if 0 < j < HD - FB + 1:
            fr[HD - j] = 1.0
            fi[HD - j] = -1.0j
        A[j] = (ifft_w @ fr).real
        Bm[j] = (ifft_w @ fi).real
    return Cr, Ci, A, Bm


def _prep(inp, n_layers=L):
    f = {k: np.asarray(v, np.float64) for k, v in inp.items()}
    Cr, Ci, A, Bm = _build_dft()

    BDb = np.zeros((D, FR))
    iBD = np.zeros((FR, D))
    for h in range(H):
        BDb[48 * h:48 * h + 48, 64 * h:64 * h + FB] = Cr
        BDb[48 * h:48 * h + 48, 64 * h + 32:64 * h + 32 + FB] = Ci
        iBD[64 * h:64 * h + FB, 48 * h:48 * h + 48] = A
        iBD[64 * h + 32:64 * h + 32 + FB, 48 * h:48 * h + 48] = Bm

    cg = f['ln1_g'].mean(1)
    assert np.abs(f['ln1_g'] - cg[:, None]).max() < 1e-12, "ln1_g must be constant/layer"
    assert np.abs(f['ln1_b'] - f['ln1_b'].mean(1)[:, None]).max() < 1e-12
    assert np.allclose(f['pe_ln_g'], 1.0) and np.allclose(f['pe_ln_b'], 0.0), "pe_ln fold"

    BD_l = np.einsum('ld,df->ldf', cg[:, None] * f['pre_g'], BDb)
    bdbias_l = np.einsum('ld,df->lf', f['pre_b'], BDb)

    aw1p = np.einsum('ld,lde->lde', cg[:, None] * f['pre_g'], f['amlp_w1']) / S
    ab1p = np.einsum('ld,lde->le', f['pre_b'], f['amlp_w1']) + f['amlp_b1']

    aw2pp = np.zeros((L, D, 2 * FR))
    ab2pp = np.zeros((L, 2 * FR))
    aw2, ab2 = f['amlp_w2'], f['amlp_b2']
    bf, bb = f['base_filter'], f['base_bias']
    for h in range(H):
        for s in range(2):
            for fq in range(FB):
                r = 64 * h + 32 * s + fq
                c0 = h * (FB * 2) + fq * 2
                wf = bf[:, h, fq][:, None] * aw2[:, :, c0]
                bf_ = bf[:, h, fq] * ab2[:, c0] + bf[:, h, fq]
                aw2pp[:, :, r] = wf
                ab2pp[:, r] = bf_
                aw2pp[:, :, FR + r] = bdbias_l[:, r][:, None] * wf
                ab2pp[:, FR + r] = bdbias_l[:, r] * bf_
                if s == 0:
                    aw2pp[:, :, FR + r] += aw2[:, :, c0 + 1]
                    ab2pp[:, FR + r] += bb[:, h, fq] + ab2[:, c0 + 1]

    w1p = np.einsum('ld,lde->lde', f['ln2_g'], f['mlp_w1'])
    b1p = np.einsum('ld,lde->le', f['ln2_b'], f['mlp_w1']) + f['mlp_b1']

    a32 = lambda x: np.ascontiguousarray(x, np.float32)
    g = {}
    g['cg'] = cg
    g['W1'] = _bf16(w1p.reshape(L, 3, 128, 4 * D).transpose(0, 2, 1, 3))            # [L,128,3,1536] bf16
    g['W2'] = _bf16(f['mlp_w2'].reshape(L, 12, 128, 3, 128).transpose(0, 2, 1, 3, 4))
    # fp8 copies (scaled by WS); W1 padded to 4 k-chunks for DoubleRow pairs
    w18 = np.zeros((L, 128, 4, 4 * D))
    w18[:, :, :3, :] = WS * w1p.reshape(L, 3, 128, 4 * D).transpose(0, 2, 1, 3)
    g['W18'] = _fp8(w18)                                                            # [L,128,4,1536]
    g['W28'] = _fp8(WS * f['mlp_w2'].reshape(L, 12, 128, 3, 128).transpose(0, 2, 1, 3, 4))
    g['BD'] = a32(BD_l.reshape(L, 3, 128, 4, 128).transpose(0, 2, 1, 3, 4))
    g['IBD'] = a32(iBD.reshape(4, 128, 3, 128).transpose(1, 0, 2, 3))
    g['AW1'] = a32(aw1p.reshape(L, 3, 128, D).transpose(0, 2, 1, 3))
    g['AB2R'] = a32(ab2pp[:, None, :])                                              # [L,1,1024]
    g['B2R'] = _bf16(f['mlp_b2'][:, None, :].reshape(L, 1, 3, 128))
    g['B2R8'] = _bf16(WS * f['mlp_b2'][:, None, :].reshape(L, 1, 3, 128))
    g['B1R'] = _bf16(b1p[:, None, :].reshape(L, 1, 12, 128))
    g['AW2'] = a32(aw2pp.reshape(L, 3, 128, 2 * FR).transpose(0, 2, 1, 3))
    # packed per-layer biases [L,128,26]: 0-2 ab1, 3-10 ab2, 11-22 b1, 23-25 b2
    bias = np.zeros((L, 128, 26))
    bias[:, :, 0:3] = ab1p.reshape(L, 3, 128).transpose(0, 2, 1)
    bias[:, :, 3:11] = ab2pp.reshape(L, 8, 128).transpose(0, 2, 1)
    bias[:, :, 11:23] = b1p.reshape(L, 12, 128).transpose(0, 2, 1)
    bias[:, :, 23:26] = f['mlp_b2'].reshape(L, 3, 128).transpose(0, 2, 1)
    g['BIAS'] = a32(bias)
    g['PEW'] = a32(f['pe_w'].reshape(3, 2, 128, 128).transpose(2, 0, 1, 3))          # [128,3,2,128]
    g['PHW'] = a32(f['phys_w'].reshape(6, 3, 128))                                   # [6,3,128]
    g['GW'] = a32(f['gate_w'].reshape(6, 128, 3, 128).transpose(1, 0, 2, 3))         # [128,6,3,128]
    fbias = np.zeros((128, 12))  # 0-2 peb, 3-5 phb, 6-8 gb, 9-11 clspe
    fbias[:, 0:3] = f['pe_b'].T
    fbias[:, 3:6] = f['phys_b'].reshape(3, 128).T
    fbias[:, 6:9] = f['gate_b'].reshape(3, 128).T
    fbias[:, 9:12] = (f['cls_token'][0, 0] + f['pos_embed'][0, 0]).reshape(3, 128).T
    g['FBIAS'] = a32(fbias)
    g['PET'] = a32(f['pos_embed'][0, 1:].T.reshape(3, 128, NP_).transpose(1, 0, 2))  # [128,3,196]
    for kk in ('norm_g', 'norm_b', 'head_w1', 'head_b1', 'head_w2', 'head_b2'):
        g[kk] = f[kk]
    g['n_layers'] = n_layers
    return g


def _bf16(x):
    import ml_dtypes
    return np.ascontiguousarray(np.asarray(x, np.float32), dtype=ml_dtypes.bfloat16)


def _fp8(x):
    import ml_dtypes
    return np.ascontiguousarray(np.asarray(x, np.float32), dtype=ml_dtypes.float8_e4m3)


def _build(g):
    import math
    n_layers = g['n_layers']
    nc = bacc.Bacc('TRN2', target_bir_lowering=False, debug=False)
    for val in (EPS,):
        t = nc.alloc_sbuf_tensor(f"const-f32-{val}", [128, 1], F32)
        nc.gpsimd.memset(t.ap(), val)
        nc.const_aps.aps[(F32, val)] = t.ap()
    nc.all_engine_barrier()

    di = lambda name, shape, dt: nc.dram_tensor(name, list(shape), dt, kind="ExternalInput")
    PATd = di('patt', (128, 3, 2, NBP), F32R)
    PFT = di('pft', (6, NBP), F32R)
    W1d = di('w1', (L, 128, 3, 1536), BF16)
    W2d = di('w2', (L, 128, 12, 3, 128), BF16)
    W18d = di('w18', (L, 128, 4, 1536), FP8)
    W28d = di('w28', (L, 128, 12, 3, 128), FP8)
    BDd = di('bd', (L, 128, 3, 4, 128), F32R)
    IBDd = di('ibd', (128, 4, 3, 128), F32R)
    AW1d = di('aw1', (L, 128, 3, 384), F32)
    AW2d = di('aw2', (L, 128, 3, 1024), F32)
    BIASd = di('bias', (L, 128, 26), F32)
    AB2Rd = di('ab2r', (L, 1, 1024), F32)
    B2Rd = di('b2r', (L, 1, 3, 128), BF16)
    B2R8d = di('b2r8', (L, 1, 3, 128), BF16)
    B1Rd = di('b1r', (L, 1, 12, 128), BF16)
    ONFd = di('onesf', (1, BC), F32)
    ONBd = di('onesb', (1, TT), BF16)
    PEWd = di('pew', (128, 3, 2, 128), F32R)
    PHWd = di('phw', (6, 3, 128), F32R)
    GWd = di('gw', (128, 6, 3, 128), F32R)
    FBIASd = di('fbias', (128, 12), F32)
    PETd = di('pet', (128, 3, NP_), F32)
    ONESd = di('ones', (128, 128), F32R)
    HCLS = nc.dram_tensor('hcls', [128, 3, BC], F32, kind="ExternalOutput")

    with tile.TileContext(nc) as tc:
        with (
            tc.tile_pool(name='const', bufs=1) as cp,
            tc.tile_pool(name='persist', bufs=1) as pp,
            tc.tile_pool(name='hnp', bufs=1) as hnp,
            tc.tile_pool(name='xqp', bufs=4) as xqp,
            tc.tile_pool(name='stp', bufs=4) as stp,
            tc.tile_pool(name='psp', bufs=6, space='PSUM') as psp,
        ):
            ones_t = cp.tile([128, 128], F32R, name='ones_t')
            nc.sync.dma_start(ones_t[:], ONESd[:])
            ibd_t = cp.tile([128, 4, 3, 128], F32R, name='ibd_t')
            nc.sync.dma_start(ibd_t[:], IBDd[:])
            onesf_t = cp.tile([1, BC], F32, name='onesf_t')
            nc.sync.dma_start(onesf_t[:], ONFd[:])
            onesb_t = cp.tile([1, TT], BF16, name='onesb_t')
            nc.sync.dma_start(onesb_t[:], ONBd[:])
            fbias_t = cp.tile([128, 12], F32, name='fbias_t')
            nc.sync.dma_start(fbias_t[:], FBIASd[:])
            pet_t = cp.tile([128, 3, NP_], F32, name='pet_t')
            nc.sync.dma_start(pet_t[:], PETd[:])

            hT = pp.tile([128, 3, NTOK], F32, name='hT')

            def stats_pre(srcs, tlen, cgl=None, pstag='ps', on_act=False):
                """LN stats (up to 1/ve) for one token tile; srcs = 3
                [128,tlen] f32 APs. Double-LN folds to a single rsqrt:
                rs1*rs2 = rsqrt((cg^2+eps)*v + eps^2). Act-table-free:
                the Sqrt is emitted separately by stats_sqrt."""
                xq = xqp.tile([128, 3, TT], F32R, tag='xq', name='xq')
                for c in range(3):
                    eng = nc.vector if c == 0 else nc.gpsimd
                    eng.tensor_mul(xq[:, c, :tlen], srcs[c], srcs[c])
                ps_s = psp.tile([128, TT], F32, tag='ps2', bufs=2, name='ps_s')
                ps_q = psp.tile([128, TT], F32, tag='ps2', bufs=2, name='ps_q')
                for c in range(3):
                    nc.tensor.matmul(ps_s[:, :tlen], ones_t[:], srcs[c].bitcast(F32R),
                                     start=(c == 0), stop=(c == 2))
                for c in range(3):
                    nc.tensor.matmul(ps_q[:, :tlen], ones_t[:], xq[:, c, :tlen],
                                     start=(c == 0), stop=(c == 2))
                if cgl is None:
                    A, Bc_ = 1.0, EPS
                else:
                    A = float(cgl) * float(cgl) + EPS
                    Bc_ = EPS * EPS
                st = stp.tile([128, 5, TT], F32, tag='st', name='st')
                m = st[:, 0, :tlen]
                rsd = st[:, 1, :tlen]
                mm = st[:, 2, :tlen]
                t1 = st[:, 3, :tlen]
                ve = st[:, 4, :tlen]
                u = st[:, 3, :tlen]  # t1's row; t1 is dead once ve is formed
                # PSUM readers must be DVE/Act (GPSIMD cannot access PSUM).
                # on_act runs the PSUM-consuming stats on the Act engine
                # (Copy/Square live in every act table): used for the
                # next-layer stats at the tail of a layer, where Act idles
                # and early PSUM reads unblock the next layer's matmuls.
                if on_act:
                    nc.scalar.activation(m, ps_s[:, :tlen], AF.Copy, scale=1.0 / D)
                    nc.scalar.activation(t1, ps_q[:, :tlen], AF.Copy, bias=Bc_,
                                         scale=A / D)
                    nc.scalar.activation(mm, ps_s[:, :tlen], AF.Square,
                                         scale=math.sqrt(A) / D)
                    nc.vector.scalar_tensor_tensor(ve, mm, -1.0, t1, ALU.mult, ALU.add)
                else:
                    # B (eps^2 for the folded double-LN, eps for LN2) is
                    # negligible vs ve ~ A*var = O(1): drop it and save an op.
                    nc.vector.tensor_scalar(m, ps_s[:, :tlen], 1.0 / D, None, ALU.mult)
                    nc.vector.scalar_tensor_tensor(mm, m, A, m, ALU.mult, ALU.mult)
                    nc.vector.scalar_tensor_tensor(ve, ps_q[:, :tlen], A / D, mm,
                                                   ALU.mult, ALU.subtract)
                return st

            def stats_sqrt(st, tlen, gate=None):
                # rsd = 1/sqrt(ve) in one table op (ve > 0 so abs is free);
                # replaces vector.reciprocal + Act Sqrt
                if gate is None:
                    nc.scalar.activation(st[:, 1, :tlen], st[:, 4, :tlen],
                                         AF.Abs_reciprocal_sqrt)
                else:
                    nc.scalar.activation(st[:, 1, :tlen], st[:, 4, :tlen],
                                         AF.Abs_reciprocal_sqrt, scale=gate)

            def sqrt_gang(sts_list, tlen):
                """Emit the sqrts of a stats batch gated on the LAST tile's
                recip, so the Act-engine scheduler runs them back-to-back
                (one sqrt<->gelu table swap per batch instead of one per
                tile)."""
                gate = stp.tile([128, 1], F32, tag='gate', name='gate', bufs=4)
                last_u = sts_list[-1][:, 3, 0:1]
                nc.vector.tensor_scalar(gate, last_u, 0.0, 1.0, ALU.mult, ALU.add)
                for st in sts_list:
                    stats_sqrt(st, tlen, gate=gate)

            def mlp_tile8(sl, h2, w18_t, w28_t, b2r8_t, onesb_t, bias_t):
                """fp8 DoubleRow MLP for one 2-sample token tile; the two cls
                columns of the residual are left to the bf16 cls path."""
                mid = midp.tile([128, 12, TT], FP8, tag='mid', name='mid')
                for grp in range(3):
                    pss = []
                    for mci in range(4):
                        mc = grp * 4 + mci
                        ps_m = psp.tile([128, TT], F32, tag='ps', name='ps_m')
                        for j in range(2):
                            nc.tensor.matmul(
                                ps_m[:], w18_t[:, 2 * j:2 * j + 2, mc * 128:(mc + 1) * 128],
                                h2[:, 2 * j:2 * j + 2, :], start=(j == 0), stop=(j == 1),
                                perf_mode=DR)
                        pss.append((mc, ps_m))
                    for mc, ps_m in pss:
                        nc.scalar.activation(mid[:, mc, :], ps_m[:], AF.Gelu,
                                             scale=IWS, bias=bias_t[:, 11 + mc:12 + mc])
                for mc in range(3):
                    ps_o = psp.tile([128, TT], F32, tag='ps', name='ps_o')
                    for j in range(6):
                        nc.tensor.matmul(ps_o[:], w28_t[:, 2 * j:2 * j + 2, mc, :],
                                         mid[:, 2 * j:2 * j + 2, :],
                                         start=(j == 0), stop=False, perf_mode=DR)
                    nc.tensor.matmul(ps_o[:], b2r8_t[:, mc, :], onesb_t[0:1, :TT],
                                     start=False, stop=True)
                    nc.vector.scalar_tensor_tensor(
                        hT[:, mc, sl].bitcast(F32R), ps_o[:], IWS,
                        hT[:, mc, sl], ALU.mult, ALU.add)
                    pcls = ps_o.rearrange("p (j s) -> p j s", s=S)[:, :, 0]
                    htc = hT[:, mc, sl].rearrange("p (j s) -> p j s", s=S)[:, :, 0]
                    nc.vector.scalar_tensor_tensor(
                        htc.bitcast(F32R), pcls, -IWS, htc, ALU.mult, ALU.add)

            # ================= front (streamed per 2-sample group) ==========
            with (
                tc.tile_pool(name='fgrp', bufs=2) as fg_,
                tc.tile_pool(name='fw', bufs=1) as fw,
            ):
                pft_t = fw.tile([6, NBP], F32R, name='pft_t')
                nc.sync.dma_start(pft_t[:], PFT[:])
                pew_t = fw.tile([128, 3, 2, 128], F32R, name='pew_t')
                nc.sync.dma_start(pew_t[:], PEWd[:])
                phw_t = fw.tile([6, 3, 128], F32R, name='phw_t')
                nc.sync.dma_start(phw_t[:], PHWd[:])
                for grp in range(4):
                    sl = slice(grp * BT, (grp + 1) * BT)
                    patg = fg_.tile([128, 3, 2, BT], F32R, tag='patg', name='patg')
                    for c in range(3):
                        nc.sync.dma_start(patg[:, c], PATd[:, c, :, sl])
                    ximg = fg_.tile([128, 3, BT], F32R, tag='ximg', name='ximg')
                    xn = fg_.tile([128, 3, BT], F32R, tag='xn', name='xn')
                    xp = fg_.tile([128, 3, BT], F32R, tag='xp', name='xp')
                    gt = fg_.tile([128, 3, BT], F32, tag='gt', name='gt')
                    for c in range(3):
                        ps_pe = psp.tile([128, TT], F32, tag='ps', name='ps_pe')
                        for kc in range(2):
                            nc.tensor.matmul(ps_pe[:, :BT], pew_t[:, c, kc, :], patg[:, c, kc, :],
                                             start=(kc == 0), stop=(kc == 1))
                        nc.scalar.activation(ximg[:, c, :], ps_pe[:, :BT], AF.Identity,
                                             bias=fbias_t[:, c:c + 1])
                    if grp == 0:
                        gw_t = fw.tile([128, 6, 3, 128], F32R, name='gw_t')
                        nc.sync.dma_start(gw_t[:], GWd[:])
                    xi = [ximg[:, c, :].bitcast(F32) for c in range(3)]
                    st = stats_pre(xi, BT)
                    stats_sqrt(st, BT)
                    m = st[:, 0, :BT]
                    rsd = st[:, 1, :BT]
                    for c in range(3):
                        eng = nc.gpsimd if c == 2 else nc.vector
                        tm = st[:, 2 + c, :BT]
                        eng.tensor_sub(tm, xi[c], m)
                        eng.tensor_mul(xn[:, c, :], tm, rsd)
                    for mc in range(3):
                        ps_ph = psp.tile([128, TT], F32, tag='ps', name='ps_ph')
                        nc.tensor.matmul(ps_ph[:, :BT], phw_t[:, mc, :], pft_t[:, sl],
                                         start=True, stop=True)
                        nc.scalar.activation(xp[:, mc, :], ps_ph[:, :BT], AF.Identity,
                                             bias=fbias_t[:, 3 + mc:4 + mc])
                    for mc in range(3):
                        ps_g = psp.tile([128, TT], F32, tag='ps', name='ps_g')
                        for kc in range(6):
                            rhs = xn[:, kc, :] if kc < 3 else xp[:, kc - 3, :]
                            nc.tensor.matmul(ps_g[:, :BT], gw_t[:, kc, mc, :], rhs,
                                             start=(kc == 0), stop=(kc == 5))
                        nc.scalar.activation(gt[:, mc, :], ps_g[:, :BT], AF.Sigmoid,
                                             bias=fbias_t[:, 6 + mc:7 + mc])
                    for bl in range(2):
                        b = 2 * grp + bl
                        psl = slice(bl * NP_, (bl + 1) * NP_)
                        tsl = slice(b * S + 1, (b + 1) * S)
                        dd = stp.tile([128, 5, TT], F32, tag='st', name='fd')
                        dv = dd[:, 0:3, :NP_]
                        nc.vector.tensor_sub(dv, xn[:, :, psl].bitcast(F32), xp[:, :, psl].bitcast(F32))
                        nc.vector.tensor_mul(dv, gt[:, :, psl], dv)
                        nc.vector.tensor_add(dv, dv, xp[:, :, psl].bitcast(F32))
                        nc.vector.tensor_add(hT[:, :, tsl].bitcast(F32R), dv, pet_t[:])
                        nc.vector.tensor_copy(hT[:, :, b * S:b * S + 1].bitcast(F32R),
                                              fbias_t[:, 9:12].unsqueeze(2))

            # ========================= transformer layers ===================
            with (
                tc.tile_pool(name='w1bp', bufs=1) as w1bp,
                tc.tile_pool(name='w2bp', bufs=1) as w2bp,
                tc.tile_pool(name='w18p', bufs=2) as w18p,
                tc.tile_pool(name='w28p', bufs=2) as w28p,
                tc.tile_pool(name='wps', bufs=1) as wps,
                tc.tile_pool(name='fgp', bufs=2) as fgp,
                tc.tile_pool(name='midp', bufs=2) as midp,
                tc.tile_pool(name='h2p', bufs=4) as h2p,
                tc.tile_pool(name='clsp', bufs=2) as clsp,
                tc.tile_pool(name='amp', bufs=1) as amp,
            ):
                pending = None
                for l in range(n_layers):
                    w1_t = w1bp.tile([128, 3, 1536], BF16, tag='w1b', name='w1_t')
                    nc.sync.dma_start(w1_t[:], W1d[l])
                    w2_t = w2bp.tile([128, 12, 3, 128], BF16, tag='w2b', name='w2_t')
                    nc.sync.dma_start(w2_t[:], W2d[l])
                    w18_t = w18p.tile([128, 4, 1536], FP8, tag='w18', name='w18_t')
                    nc.sync.dma_start(w18_t[:], W18d[l])
                    w28_t = w28p.tile([128, 12, 3, 128], FP8, tag='w28', name='w28_t')
                    nc.sync.dma_start(w28_t[:], W28d[l])
                    bd_t = wps.tile([128, 3, 4, 128], F32R, tag='bd', name='bd_t')
                    nc.sync.dma_start(bd_t[:], BDd[l])
                    aw1_t = wps.tile([128, 3, 384], F32, tag='aw1', name='aw1_t')
                    nc.sync.dma_start(aw1_t[:], AW1d[l])
                    aw2_t = wps.tile([128, 3, 1024], F32, tag='aw2', name='aw2_t')
                    nc.sync.dma_start(aw2_t[:], AW2d[l])
                    bias_t = wps.tile([128, 26], F32, tag='bias', name='bias_t')
                    nc.sync.dma_start(bias_t[:], BIASd[l])
                    ab2r_t = wps.tile([1, 1024], F32, tag='ab2r', name='ab2r_t')
                    nc.sync.dma_start(ab2r_t[:], AB2Rd[l])
                    b2r_t = wps.tile([1, 3, 128], BF16, tag='b2r', name='b2r_t')
                    nc.sync.dma_start(b2r_t[:], B2Rd[l])
                    b2r8_t = wps.tile([1, 3, 128], BF16, tag='b2r8', name='b2r8_t')
                    nc.sync.dma_start(b2r8_t[:], B2R8d[l])
                    b1r_t = wps.tile([1, 12, 128], BF16, tag='b1r', name='b1r_t')
                    nc.sync.dma_start(b1r_t[:], B1Rd[l])

                    hn = hnp.tile([128, 3, NTOK], F32R, tag='hn', name='hn')
                    mh = amp.tile([128, 3, BC], F32, tag='mh', name='mh')
                    if pending is None:
                        sts = []
                        for t in range(4):
                            sl = slice(t * TT, (t + 1) * TT)
                            hs = [hT[:, c, sl] for c in range(3)]
                            sts.append((sl, hs, stats_pre(hs, TT, cgl=g['cg'][l])))
                        sqrt_gang([sts[t][2] for t in range(4)], TT)
                    else:
                        sts = pending
                    ps_u = psp.tile([128, TT], F32, tag='ps2', bufs=2, name='ps_u')
                    ps_e = psp.tile([128, TT], F32, tag='ps2', bufs=2, name='ps_e')
                    u2t = amp.tile([128, 3, BC], F32, tag='u2', name='u2t')
                    eff = amp.tile([128, 8, BC], F32, tag='eff', name='eff')
                    for t in range(4):
                        sl, hs, st = sts[t]
                        m = st[:, 0, :]
                        rsd = st[:, 1, :]
                        # LN1 apply with fused per-sample token-sum (-> mh)
                        for c in range(3):
                            tm = st[:, 2 + c, :]
                            eng = nc.gpsimd if c == 2 else nc.vector
                            eng.tensor_sub(tm, hs[c], m)
                            for j in range(2):
                                jsl = slice(j * S, (j + 1) * S)
                                nc.vector.scalar_tensor_tensor(
                                    hn[:, c, sl][:, jsl], tm[:, jsl], 1.0,
                                    rsd[:, jsl], ALU.mult, ALU.mult,
                                    accum_out=mh[:, c, 2 * t + j:2 * t + j + 1])
                        bsl = slice(2 * t, 2 * t + 2)
                        for mc in range(3):
                            for kc in range(3):
                                nc.tensor.matmul(
                                    ps_u[:, mc * BC:mc * BC + BC][:, bsl],
                                    aw1_t[:, kc, mc * 128:(mc + 1) * 128],
                                    mh[:, kc, bsl], start=(kc == 0), stop=(kc == 2))
                        if t in (1, 3):
                            hsl = slice(0, 4) if t == 1 else slice(4, 8)
                            for mc in range(3):
                                nc.scalar.activation(u2t[:, mc, hsl],
                                                     ps_u[:, mc * BC:mc * BC + BC][:, hsl],
                                                     AF.Gelu, bias=bias_t[:, mc:mc + 1])
                            for mt in range(8):
                                for kc in range(3):
                                    nc.tensor.matmul(
                                        ps_e[:, mt * BC:mt * BC + BC][:, hsl],
                                        aw2_t[:, kc, mt * 128:(mt + 1) * 128],
                                        u2t[:, kc, hsl], start=(kc == 0), stop=False)
                                nc.tensor.matmul(
                                    ps_e[:, mt * BC:mt * BC + BC][:, hsl],
                                    ab2r_t[:, mt * 128:(mt + 1) * 128],
                                    onesf_t[0:1, hsl], start=False, stop=True)
                                nc.vector.tensor_scalar(eff[:, mt, hsl],
                                                        ps_e[:, mt * BC:mt * BC + BC][:, hsl],
                                                        1.0, None, ALU.mult)  # PSUM read: DVE

                    # FFT mixer
                    KCS_F = [[0], [0, 1], [1, 2], [2]]
                    KCS_I = [[0, 1], [1, 2], [2, 3]]
                    for t in range(4):
                        sl = slice(t * TT, (t + 1) * TT)
                        fg = fgp.tile([128, 4, TT], F32R, tag='fg', name='fg')
                        for mc in range(4):
                            ps_F = psp.tile([128, TT], F32, tag='ps', name='ps_F')
                            kcs = KCS_F[mc]
                            for i, kc in enumerate(kcs):
                                nc.tensor.matmul(ps_F[:], bd_t[:, kc, mc, :], hn[:, kc, sl],
                                                 start=(i == 0), stop=(i == len(kcs) - 1))
                            for j in range(2):
                                bb = 2 * t + j
                                nc.scalar.activation(fg[:, mc, j * S:(j + 1) * S],
                                                     ps_F[:, j * S:(j + 1) * S], AF.Gelu,
                                                     scale=eff[:, mc, bb:bb + 1],
                                                     bias=eff[:, 4 + mc, bb:bb + 1])
                        for mc in range(3):
                            ps_A = psp.tile([128, TT], F32, tag='ps', name='ps_A')
                            kcs = KCS_I[mc]
                            for i, kc in enumerate(kcs):
                                nc.tensor.matmul(ps_A[:], ibd_t[:, kc, mc, :], fg[:, kc, :],
                                                 start=(i == 0), stop=(i == len(kcs) - 1))
                            nc.vector.tensor_add(hT[:, mc, sl].bitcast(F32R), hT[:, mc, sl], ps_A[:])

                    # LN2 stats (sqrt batched after all 4 tiles)
                    ln2q = []
                    for t in range(4):
                        sl = slice(t * TT, (t + 1) * TT)
                        hs = [hT[:, c, sl] for c in range(3)]
                        ln2q.append((sl, hs, stats_pre(hs, TT)))
                        if t in (1, 3):
                            sqrt_gang([ln2q[t - 1][2], ln2q[t][2]], TT)

                    # LN2 apply -> h2 fp8 (+ bf16 cls columns)
                    h2cls = clsp.tile([128, 3, BC], BF16, tag='h2c', name='h2cls')
                    h2s = []
                    for t in range(4):
                        sl, hs, st = ln2q[t]
                        m = st[:, 0, :]
                        rsd = st[:, 1, :]
                        h2 = h2p.tile([128, 4, TT], FP8, tag='h2', name='h2')
                        nc.gpsimd.memset(h2[:, 3, :], 0.0)
                        for c in range(3):
                            tm = st[:, 2 + c, :]
                            eng = nc.gpsimd if c == 2 else nc.vector
                            eng.tensor_sub(tm, hs[c], m)
                            nc.gpsimd.tensor_mul(h2[:, c, :], tm, rsd)
                            tm2 = tm.rearrange("p (j s) -> p j s", s=S)[:, :, 0]
                            rs2 = rsd.rearrange("p (j s) -> p j s", s=S)[:, :, 0]
                            nc.vector.scalar_tensor_tensor(h2cls[:, c, 2 * t:2 * t + 2],
                                                           tm2, 1.0, rs2, ALU.mult, ALU.mult)
                        h2s.append(h2)

                    # cls-column MLP in bf16 (8 columns, all samples at once)
                    midcls = clsp.tile([128, 12, BC], BF16, tag='midc', name='midcls')
                    ps_c1 = psp.tile([128, 12 * BC], F32, tag='ps', name='ps_c1')
                    for mc in range(12):
                        oc = ps_c1[:, mc * BC:(mc + 1) * BC]
                        for kc in range(3):
                            nc.tensor.matmul(oc, w1_t[:, kc, mc * 128:(mc + 1) * 128],
                                             h2cls[:, kc, :], start=(kc == 0), stop=False)
                        nc.tensor.matmul(oc, b1r_t[:, mc, :], onesb_t[0:1, :BC],
                                         start=False, stop=True)
                    nc.scalar.activation(midcls[:].rearrange("p a b -> p (a b)"), ps_c1[:],
                                         AF.Gelu)
                    ps_c2 = psp.tile([128, 3 * BC], F32, tag='ps', name='ps_c2')
                    for mc in range(3):
                        oc = ps_c2[:, mc * BC:(mc + 1) * BC]
                        for kc in range(12):
                            nc.tensor.matmul(oc, w2_t[:, kc, mc, :], midcls[:, kc, :],
                                             start=(kc == 0), stop=False)
                        nc.tensor.matmul(oc, b2r_t[:, mc, :], onesb_t[0:1, :BC],
                                         start=False, stop=True)
                    for mc in range(3):
                        ht_cls = hT[:, mc, :].rearrange("p (b s) -> p b s", s=S)[:, :, 0]
                        nc.vector.scalar_tensor_tensor(ht_cls.bitcast(F32R), ps_c2[:, mc * BC:(mc + 1) * BC],
                                                       1.0, ht_cls, ALU.mult, ALU.add)

                    # fp8 MLP (patch tokens; cls columns of the residual are
                    # skipped — the bf16 path above owns them)
                    nxt = []
                    for t in range(4):
                        mlp_tile8(ln2q[t][0], h2s[t], w18_t, w28_t, b2r8_t,
                                  onesb_t, bias_t)
                        sl, hs, _ = ln2q[t]
                        if l + 1 < n_layers:
                            nxt.append((sl, hs, stats_pre(hs, TT, cgl=g['cg'][l + 1])))
                            if t in (1, 3):
                                sqrt_gang([nxt[t - 1][2], nxt[t][2]], TT)
                    pending = nxt if l + 1 < n_layers else None

                out_ap = hT[:].rearrange("p c (b s) -> p c b s", s=S)[:, :, :, 0]
                nc.sync.dma_start(HCLS[:], out_ap)

    nc.compile()
    return nc


def _gelu_np(x):
    try:
        from scipy.special import erf
    except ImportError:
        import math
        erf = np.vectorize(math.erf)
    return x * 0.5 * (1.0 + erf(x / np.sqrt(2.0)))


def _head(hcls, g):
    x = hcls.astype(np.float64).T
    m = x.mean(1, keepdims=True)
    v = ((x - m) ** 2).mean(1, keepdims=True)
    cls = (x - m) / np.sqrt(v + EPS) * g['norm_g'] + g['norm_b']
    u = _gelu_np(cls @ g['head_w1'] + g['head_b1'])
    return ((u @ g['head_w2'])[:, 0] + g['head_b2'][0]).astype(np.float32)


def _in_maps(inputs, g):
    x = np.ascontiguousarray(inputs['x'], np.float32)
    pf = np.ascontiguousarray(inputs['patch_feats'], np.float32)
    shared = dict(
        w1=g['W1'], w2=g['W2'], w18=g['W18'], w28=g['W28'], bd=g['BD'],
        ibd=g['IBD'], aw1=g['AW1'],
        aw2=g['AW2'], bias=g['BIAS'], ab2r=g['AB2R'], b2r=g['B2R'],
        b2r8=g['B2R8'], b1r=g['B1R'],
        onesf=np.ones((1, BC), np.float32),
        onesb=_bf16(np.ones((1, TT))), pew=g['PEW'], phw=g['PHW'], gw=g['GW'],
        fbias=g['FBIAS'], pet=g['PET'],
        ones=np.ones((128, 128), np.float32),
    )
    Hp = 224 // P
    pat = x.reshape(B, 3, Hp, P, Hp, P).transpose(0, 1, 2, 4, 3, 5).reshape(B, 3, NP_, 2, 128)
    maps = []
    for i in range(NCORES):
        m = dict(shared)
        pc = pat[i * BC:(i + 1) * BC]                       # [BC,3,196,2,128]
        m['patt'] = np.ascontiguousarray(pc.transpose(4, 1, 3, 0, 2).reshape(128, 3, 2, NBP))
        m['pft'] = np.ascontiguousarray(pf[i * BC:(i + 1) * BC].reshape(NBP, 6).T)
        maps.append(m)
    return maps


def kernel(**inputs):
    inputs = {k: np.asarray(v) for k, v in inputs.items()}
    g = _prep(inputs)
    # program structure bakes per-layer ln1 gains into immediates; key on them
    key = (tuple(np.round(np.asarray(g['cg'], np.float64), 12)),)
    if _CACHE.get('key') != key:
        _CACHE['prog'] = _build(g)
        _CACHE['key'] = key
    nc = _CACHE['prog']
    res = run_bass_kernel_spmd(nc, _in_maps(inputs, g), list(range(NCORES)))
    _CACHE['last_res'] = res
    _CACHE['last_g'] = g
    hcls = np.concatenate(
        [r['hcls'].transpose(1, 0, 2).reshape(D, BC) for r in res.results], axis=1)
    return _head(hcls, g)


if __name__ == '__main__':
    d = np.load('/root/problem/ref_data.npz')
    inputs = {k: d[k] for k in d.files if k != 'expected'}
    y = kernel(**inputs)
    exp = d['expected']
    err = np.abs(y - exp)
    print("max abs err:", err.max())
    print("Relative error:", err.max() / np.abs(exp).max())


# revision 48
# speedup vs baseline: 1.0170x; 1.0170x over previous
"""Trainium2 Bass kernel for nn_FFTPermeabilityPredictorPatchPhysics.

Sharding: pure data parallel — 8 samples per NeuronCore, weights replicated.
On-device layout: residual stream transposed, hT [3x128 d-chunks, 1576 tok],
kept in SBUF for all 12 layers. FFT/iFFT as block-diagonal matmuls over a
512-row padded frequency layout (head h -> rows 64h+32s+f). LN stats via
ones-matmul partition reductions broadcast to all partitions; the adaptive
spectral filter is fused into the ACT-engine gelu via per-partition
scale/bias. The MLP runs fp8e4 DoubleRow (K=256 per instruction) for the
196 patch tokens with weight scale 64 folded into the gelu scale and the
residual scalar_tensor_tensor; the cls token column (which feeds the head
directly, without the 1/197 mean dilution of patch tokens) is recomputed
in bf16 against the same-layer bf16 weights. All weight folding done
host-side in numpy: double-LN collapse, pre_g/ln2_g into following
matmuls, base_filter and (1+ap) into amlp_w2, 1/197 token-mean into
amlp_w1, DFT matrices baked. Final LN + head on the 64 cls vectors runs
host-side in float64.
"""
import numpy as np

import concourse.bacc as bacc
import concourse.mybir as mybir
import concourse.tile as tile
from concourse.bass_utils import run_bass_kernel_spmd

F32 = mybir.dt.float32
F32R = mybir.dt.float32r
BF16 = mybir.dt.bfloat16
FP8 = mybir.dt.float8e4
AF = mybir.ActivationFunctionType
ALU = mybir.AluOpType
DR = mybir.MatmulPerfMode.DoubleRow

B, D, H, HD, FB, S, L, P, NP_ = 64, 384, 8, 48, 25, 197, 12, 16, 196
EPS = 1e-5
FR = 512
NCORES = 8
BC = B // NCORES     # 8 samples/core
NTOK = BC * S        # 1576
TT = 394             # token tile = 2 samples
NBP = BC * NP_       # 1568
BT = 392             # patch tile = 2 samples
WS = 64.0            # fp8 weight scale for both MLP matmuls
IWS = 1.0 / WS

_CACHE = {}


def _build_dft():
    n = np.arange(HD)
    k = np.arange(FB)
    ang = -2 * np.pi * np.outer(n, k) / HD
    Cr = np.cos(ang) / np.sqrt(HD)
    Ci = np.sin(ang) / np.sqrt(HD)
    A = np.zeros((FB, HD))
    Bm = np.zeros((FB, HD))
    ifft_w = np.exp(2j * np.pi * np.outer(np.arange(HD), np.arange(HD)) / HD) / np.sqrt(HD)
    for j in range(FB):
        fr = np.zeros(HD, complex)
        fi = np.zeros(HD, complex)
        fr[j] = 1.0
        fi[j] = 1.0j
        if 0 < j < HD - FB + 1:
            fr[HD - j] = 1.0
            fi[HD - j] = -1.0j
        A[j] = (ifft_w @ fr).real
        Bm[j] = (ifft_w @ fi).real
    return Cr, Ci, A, Bm


def _prep(inp, n_layers=L):
    f = {k: np.asarray(v, np.float64) for k, v in inp.items()}
    Cr, Ci, A, Bm = _build_dft()

    BDb = np.zeros((D, FR))
    iBD = np.zeros((FR, D))
    for h in range(H):
        BDb[48 * h:48 * h + 48, 64 * h:64 * h + FB] = Cr
        BDb[48 * h:48 * h + 48, 64 * h + 32:64 * h + 32 + FB] = Ci
        iBD[64 * h:64 * h + FB, 48 * h:48 * h + 48] = A
        iBD[64 * h + 32:64 * h + 32 + FB, 48 * h:48 * h + 48] = Bm

    cg = f['ln1_g'].mean(1)
    assert np.abs(f['ln1_g'] - cg[:, None]).max() < 1e-12, "ln1_g must be constant/layer"
    assert np.abs(f['ln1_b'] - f['ln1_b'].mean(1)[:, None]).max() < 1e-12
    assert np.allclose(f['pe_ln_g'], 1.0) and np.allclose(f['pe_ln_b'], 0.0), "pe_ln fold"

    BD_l = np.einsum('ld,df->ldf', cg[:, None] * f['pre_g'], BDb)
    bdbias_l = np.einsum('ld,df->lf', f['pre_b'], BDb)

    aw1p = np.einsum('ld,lde->lde', cg[:, None] * f['pre_g'], f['amlp_w1']) / S
    ab1p = np.einsum('ld,lde->le', f['pre_b'], f['amlp_w1']) + f['amlp_b1']

    aw2pp = np.zeros((L, D, 2 * FR))
    ab2pp = np.zeros((L, 2 * FR))
    aw2, ab2 = f['amlp_w2'], f['amlp_b2']
    bf, bb = f['base_filter'], f['base_bias']
    for h in range(H):
        for s in range(2):
            for fq in range(FB):
                r = 64 * h + 32 * s + fq
                c0 = h * (FB * 2) + fq * 2
                wf = bf[:, h, fq][:, None] * aw2[:, :, c0]
                bf_ = bf[:, h, fq] * ab2[:, c0] + bf[:, h, fq]
                aw2pp[:, :, r] = wf
                ab2pp[:, r] = bf_
                aw2pp[:, :, FR + r] = bdbias_l[:, r][:, None] * wf
                ab2pp[:, FR + r] = bdbias_l[:, r] * bf_
                if s == 0:
                    aw2pp[:, :, FR + r] += aw2[:, :, c0 + 1]
                    ab2pp[:, FR + r] += bb[:, h, fq] + ab2[:, c0 + 1]

    w1p = np.einsum('ld,lde->lde', f['ln2_g'], f['mlp_w1'])
    b1p = np.einsum('ld,lde->le', f['ln2_b'], f['mlp_w1']) + f['mlp_b1']

    a32 = lambda x: np.ascontiguousarray(x, np.float32)
    g = {}
    g['cg'] = cg
    g['W1'] = _bf16(w1p.reshape(L, 3, 128, 4 * D).transpose(0, 2, 1, 3))            # [L,128,3,1536] bf16
    g['W2'] = _bf16(f['mlp_w2'].reshape(L, 12, 128, 3, 128).transpose(0, 2, 1, 3, 4))
    # fp8 copies (scaled by WS); W1 padded to 4 k-chunks for DoubleRow pairs
    w18 = np.zeros((L, 128, 4, 4 * D))
    w18[:, :, :3, :] = WS * w1p.reshape(L, 3, 128, 4 * D).transpose(0, 2, 1, 3)
    g['W18'] = _fp8(w18)                                                            # [L,128,4,1536]
    g['W28'] = _fp8(WS * f['mlp_w2'].reshape(L, 12, 128, 3, 128).transpose(0, 2, 1, 3, 4))
    g['BD'] = a32(BD_l.reshape(L, 3, 128, 4, 128).transpose(0, 2, 1, 3, 4))
    g['IBD'] = a32(iBD.reshape(4, 128, 3, 128).transpose(1, 0, 2, 3))
    g['AW1'] = a32(aw1p.reshape(L, 3, 128, D).transpose(0, 2, 1, 3))
    g['AB2R'] = a32(ab2pp[:, None, :])                                              # [L,1,1024]
    g['B2R'] = _bf16(f['mlp_b2'][:, None, :].reshape(L, 1, 3, 128))
    g['B2R8'] = _bf16(WS * f['mlp_b2'][:, None, :].reshape(L, 1, 3, 128))
    g['B1R'] = _bf16(b1p[:, None, :].reshape(L, 1, 12, 128))
    g['AW2'] = a32(aw2pp.reshape(L, 3, 128, 2 * FR).transpose(0, 2, 1, 3))
    # packed per-layer biases [L,128,26]: 0-2 ab1, 3-10 ab2, 11-22 b1, 23-25 b2
    bias = np.zeros((L, 128, 26))
    bias[:, :, 0:3] = ab1p.reshape(L, 3, 128).transpose(0, 2, 1)
    bias[:, :, 3:11] = ab2pp.reshape(L, 8, 128).transpose(0, 2, 1)
    bias[:, :, 11:23] = b1p.reshape(L, 12, 128).transpose(0, 2, 1)
    bias[:, :, 23:26] = f['mlp_b2'].reshape(L, 3, 128).transpose(0, 2, 1)
    g['BIAS'] = a32(bias)
    g['PEW'] = a32(f['pe_w'].reshape(3, 2, 128, 128).transpose(2, 0, 1, 3))          # [128,3,2,128]
    g['PHW'] = a32(f['phys_w'].reshape(6, 3, 128))                                   # [6,3,128]
    g['GW'] = a32(f['gate_w'].reshape(6, 128, 3, 128).transpose(1, 0, 2, 3))         # [128,6,3,128]
    fbias = np.zeros((128, 12))  # 0-2 peb, 3-5 phb, 6-8 gb, 9-11 clspe
    fbias[:, 0:3] = f['pe_b'].T
    fbias[:, 3:6] = f['phys_b'].reshape(3, 128).T
    fbias[:, 6:9] = f['gate_b'].reshape(3, 128).T
    fbias[:, 9:12] = (f['cls_token'][0, 0] + f['pos_embed'][0, 0]).reshape(3, 128).T
    g['FBIAS'] = a32(fbias)
    g['PET'] = a32(f['pos_embed'][0, 1:].T.reshape(3, 128, NP_).transpose(1, 0, 2))  # [128,3,196]
    for kk in ('norm_g', 'norm_b', 'head_w1', 'head_b1', 'head_w2', 'head_b2'):
        g[kk] = f[kk]
    g['n_layers'] = n_layers
    return g


def _bf16(x):
    import ml_dtypes
    return np.ascontiguousarray(np.asarray(x, np.float32), dtype=ml_dtypes.bfloat16)


def _fp8(x):
    import ml_dtypes
    return np.ascontiguousarray(np.asarray(x, np.float32), dtype=ml_dtypes.float8_e4m3)


def _build(g):
    import math
    n_layers = g['n_layers']
    nc = bacc.Bacc('TRN2', target_bir_lowering=False, debug=False)
    for val in (EPS,):
        t = nc.alloc_sbuf_tensor(f"const-f32-{val}", [128, 1], F32)
        nc.gpsimd.memset(t.ap(), val)
        nc.const_aps.aps[(F32, val)] = t.ap()
    nc.all_engine_barrier()

    di = lambda name, shape, dt: nc.dram_tensor(name, list(shape), dt, kind="ExternalInput")
    PATd = di('patt', (128, 3, 2, NBP), F32R)
    PFT = di('pft', (6, NBP), F32R)
    W1d = di('w1', (L, 128, 3, 1536), BF16)
    W2d = di('w2', (L, 128, 12, 3, 128), BF16)
    W18d = di('w18', (L, 128, 4, 1536), FP8)
    W28d = di('w28', (L, 128, 12, 3, 128), FP8)
    BDd = di('bd', (L, 128, 3, 4, 128), F32R)
    IBDd = di('ibd', (128, 4, 3, 128), F32R)
    AW1d = di('aw1', (L, 128, 3, 384), F32)
    AW2d = di('aw2', (L, 128, 3, 1024), F32)
    BIASd = di('bias', (L, 128, 26), F32)
    AB2Rd = di('ab2r', (L, 1, 1024), F32)
    B2Rd = di('b2r', (L, 1, 3, 128), BF16)
    B2R8d = di('b2r8', (L, 1, 3, 128), BF16)
    B1Rd = di('b1r', (L, 1, 12, 128), BF16)
    ONFd = di('onesf', (1, BC), F32)
    ONBd = di('onesb', (1, TT), BF16)
    PEWd = di('pew', (128, 3, 2, 128), F32R)
    PHWd = di('phw', (6, 3, 128), F32R)
    GWd = di('gw', (128, 6, 3, 128), F32R)
    FBIASd = di('fbias', (128, 12), F32)
    PETd = di('pet', (128, 3, NP_), F32)
    ONESd = di('ones', (128, 128), F32R)
    HCLS = nc.dram_tensor('hcls', [128, 3, BC], F32, kind="ExternalOutput")

    with tile.TileContext(nc) as tc:
        with (
            tc.tile_pool(name='const', bufs=1) as cp,
            tc.tile_pool(name='persist', bufs=1) as pp,
            tc.tile_pool(name='hnp', bufs=1) as hnp,
            tc.tile_pool(name='xqp', bufs=4) as xqp,
            tc.tile_pool(name='stp', bufs=4) as stp,
            tc.tile_pool(name='psp', bufs=6, space='PSUM') as psp,
        ):
            ones_t = cp.tile([128, 128], F32R, name='ones_t')
            nc.sync.dma_start(ones_t[:], ONESd[:])
            ibd_t = cp.tile([128, 4, 3, 128], F32R, name='ibd_t')
            nc.sync.dma_start(ibd_t[:], IBDd[:])
            onesf_t = cp.tile([1, BC], F32, name='onesf_t')
            nc.sync.dma_start(onesf_t[:], ONFd[:])
            onesb_t = cp.tile([1, TT], BF16, name='onesb_t')
            nc.sync.dma_start(onesb_t[:], ONBd[:])
            fbias_t = cp.tile([128, 12], F32, name='fbias_t')
            nc.sync.dma_start(fbias_t[:], FBIASd[:])
            pet_t = cp.tile([128, 3, NP_], F32, name='pet_t')
            nc.sync.dma_start(pet_t[:], PETd[:])

            hT = pp.tile([128, 3, NTOK], F32, name='hT')

            def stats_pre(srcs, tlen, cgl=None, pstag='ps', on_act=False):
                """LN stats (up to 1/ve) for one token tile; srcs = 3
                [128,tlen] f32 APs. Double-LN folds to a single rsqrt:
                rs1*rs2 = rsqrt((cg^2+eps)*v + eps^2). Act-table-free:
                the Sqrt is emitted separately by stats_sqrt."""
                xq = xqp.tile([128, 3, TT], F32R, tag='xq', name='xq')
                for c in range(3):
                    eng = nc.vector if c == 0 else nc.gpsimd
                    eng.tensor_mul(xq[:, c, :tlen], srcs[c], srcs[c])
                ps_s = psp.tile([128, TT], F32, tag='ps2', bufs=2, name='ps_s')
                ps_q = psp.tile([128, TT], F32, tag='ps2', bufs=2, name='ps_q')
                for c in range(3):
                    nc.tensor.matmul(ps_s[:, :tlen], ones_t[:], srcs[c].bitcast(F32R),
                                     start=(c == 0), stop=(c == 2))
                for c in range(3):
                    nc.tensor.matmul(ps_q[:, :tlen], ones_t[:], xq[:, c, :tlen],
                                     start=(c == 0), stop=(c == 2))
                if cgl is None:
                    A, Bc_ = 1.0, EPS
                else:
                    A = float(cgl) * float(cgl) + EPS
                    Bc_ = EPS * EPS
                st = stp.tile([128, 5, TT], F32, tag='st', name='st')
                m = st[:, 0, :tlen]
                rsd = st[:, 1, :tlen]
                mm = st[:, 2, :tlen]
                t1 = st[:, 3, :tlen]
                ve = st[:, 4, :tlen]
                u = st[:, 3, :tlen]  # t1's row; t1 is dead once ve is formed
                # PSUM readers must be DVE/Act (GPSIMD cannot access PSUM).
                # on_act runs the PSUM-consuming stats on the Act engine
                # (Copy/Square live in every act table): used for the
                # next-layer stats at the tail of a layer, where Act idles
                # and early PSUM reads unblock the next layer's matmuls.
                if on_act:
                    nc.scalar.activation(m, ps_s[:, :tlen], AF.Copy, scale=1.0 / D)
                    nc.scalar.activation(t1, ps_q[:, :tlen], AF.Copy, bias=Bc_,
                                         scale=A / D)
                    nc.scalar.activation(mm, ps_s[:, :tlen], AF.Square,
                                         scale=math.sqrt(A) / D)
                    nc.vector.scalar_tensor_tensor(ve, mm, -1.0, t1, ALU.mult, ALU.add)
                else:
                    # B (eps^2 for the folded double-LN, eps for LN2) is
                    # negligible vs ve ~ A*var = O(1): drop it and save an op.
                    nc.vector.tensor_scalar(m, ps_s[:, :tlen], 1.0 / D, None, ALU.mult)
                    nc.vector.scalar_tensor_tensor(mm, m, A, m, ALU.mult, ALU.mult)
                    nc.vector.scalar_tensor_tensor(ve, ps_q[:, :tlen], A / D, mm,
                                                   ALU.mult, ALU.subtract)
                return st

            def stats_sqrt(st, tlen, gate=None):
                # rsd = 1/sqrt(ve) in one table op (ve > 0 so abs is free);
                # replaces vector.reciprocal + Act Sqrt
                if gate is None:
                    nc.scalar.activation(st[:, 1, :tlen], st[:, 4, :tlen],
                                         AF.Abs_reciprocal_sqrt)
                else:
                    nc.scalar.activation(st[:, 1, :tlen], st[:, 4, :tlen],
                                         AF.Abs_reciprocal_sqrt, scale=gate)

            def sqrt_gang(sts_list, tlen):
                """Emit the sqrts of a stats batch gated on the LAST tile's
                recip, so the Act-engine scheduler runs them back-to-back
                (one sqrt<->gelu table swap per batch instead of one per
                tile)."""
                gate = stp.tile([128, 1], F32, tag='gate', name='gate', bufs=4)
                last_u = sts_list[-1][:, 4, 0:1]
                nc.vector.tensor_scalar(gate, last_u, 0.0, 1.0, ALU.mult, ALU.add)
                for st in sts_list:
                    stats_sqrt(st, tlen, gate=gate)

            def mlp_tile8(sl, h2, w18_t, w28_t, b2r8_t, onesb_t, bias_t):
                """fp8 DoubleRow MLP for one 2-sample token tile; the two cls
                columns of the residual are left to the bf16 cls path."""
                mid = midp.tile([128, 12, TT], FP8, tag='mid', name='mid')
                for grp in range(3):
                    pss = []
                    for mci in range(4):
                        mc = grp * 4 + mci
                        ps_m = psp.tile([128, TT], F32, tag='ps', name='ps_m')
                        for j in range(2):
                            nc.tensor.matmul(
                                ps_m[:], w18_t[:, 2 * j:2 * j + 2, mc * 128:(mc + 1) * 128],
                                h2[:, 2 * j:2 * j + 2, :], start=(j == 0), stop=(j == 1),
                                perf_mode=DR)
                        pss.append((mc, ps_m))
                    for mc, ps_m in pss:
                        nc.scalar.activation(mid[:, mc, :], ps_m[:], AF.Gelu,
                                             scale=IWS, bias=bias_t[:, 11 + mc:12 + mc])
                for mc in range(3):
                    ps_o = psp.tile([128, TT], F32, tag='ps', name='ps_o')
                    for j in range(6):
                        nc.tensor.matmul(ps_o[:], w28_t[:, 2 * j:2 * j + 2, mc, :],
                                         mid[:, 2 * j:2 * j + 2, :],
                                         start=(j == 0), stop=False, perf_mode=DR)
                    nc.tensor.matmul(ps_o[:], b2r8_t[:, mc, :], onesb_t[0:1, :TT],
                                     start=False, stop=True)
                    nc.vector.scalar_tensor_tensor(
                        hT[:, mc, sl].bitcast(F32R), ps_o[:], IWS,
                        hT[:, mc, sl], ALU.mult, ALU.add)
                    pcls = ps_o.rearrange("p (j s) -> p j s", s=S)[:, :, 0]
                    htc = hT[:, mc, sl].rearrange("p (j s) -> p j s", s=S)[:, :, 0]
                    nc.vector.scalar_tensor_tensor(
                        htc.bitcast(F32R), pcls, -IWS, htc, ALU.mult, ALU.add)

            # ================= front (streamed per 2-sample group) ==========
            with (
                tc.tile_pool(name='fgrp', bufs=2) as fg_,
                tc.tile_pool(name='fw', bufs=1) as fw,
            ):
                pft_t = fw.tile([6, NBP], F32R, name='pft_t')
                nc.sync.dma_start(pft_t[:], PFT[:])
                pew_t = fw.tile([128, 3, 2, 128], F32R, name='pew_t')
                nc.sync.dma_start(pew_t[:], PEWd[:])
                phw_t = fw.tile([6, 3, 128], F32R, name='phw_t')
                nc.sync.dma_start(phw_t[:], PHWd[:])
                for grp in range(4):
                    sl = slice(grp * BT, (grp + 1) * BT)
                    patg = fg_.tile([128, 3, 2, BT], F32R, tag='patg', name='patg')
                    for c in range(3):
                        nc.sync.dma_start(patg[:, c], PATd[:, c, :, sl])
                    ximg = fg_.tile([128, 3, BT], F32R, tag='ximg', name='ximg')
                    xn = fg_.tile([128, 3, BT], F32R, tag='xn', name='xn')
                    xp = fg_.tile([128, 3, BT], F32R, tag='xp', name='xp')
                    gt = fg_.tile([128, 3, BT], F32, tag='gt', name='gt')
                    for c in range(3):
                        ps_pe = psp.tile([128, TT], F32, tag='ps', name='ps_pe')
                        for kc in range(2):
                            nc.tensor.matmul(ps_pe[:, :BT], pew_t[:, c, kc, :], patg[:, c, kc, :],
                                             start=(kc == 0), stop=(kc == 1))
                        nc.scalar.activation(ximg[:, c, :], ps_pe[:, :BT], AF.Identity,
                                             bias=fbias_t[:, c:c + 1])
                    if grp == 0:
                        gw_t = fw.tile([128, 6, 3, 128], F32R, name='gw_t')
                        nc.sync.dma_start(gw_t[:], GWd[:])
                    xi = [ximg[:, c, :].bitcast(F32) for c in range(3)]
                    st = stats_pre(xi, BT)
                    stats_sqrt(st, BT)
                    m = st[:, 0, :BT]
                    rsd = st[:, 1, :BT]
                    for c in range(3):
                        eng = nc.gpsimd if c == 2 else nc.vector
                        tm = st[:, 2 + c, :BT]
                        eng.tensor_sub(tm, xi[c], m)
                        eng.tensor_mul(xn[:, c, :], tm, rsd)
                    for mc in range(3):
                        ps_ph = psp.tile([128, TT], F32, tag='ps', name='ps_ph')
                        nc.tensor.matmul(ps_ph[:, :BT], phw_t[:, mc, :], pft_t[:, sl],
                                         start=True, stop=True)
                        nc.scalar.activation(xp[:, mc, :], ps_ph[:, :BT], AF.Identity,
                                             bias=fbias_t[:, 3 + mc:4 + mc])
                    for mc in range(3):
                        ps_g = psp.tile([128, TT], F32, tag='ps', name='ps_g')
                        for kc in range(6):
                            rhs = xn[:, kc, :] if kc < 3 else xp[:, kc - 3, :]
                            nc.tensor.matmul(ps_g[:, :BT], gw_t[:, kc, mc, :], rhs,
                                             start=(kc == 0), stop=(kc == 5))
                        nc.scalar.activation(gt[:, mc, :], ps_g[:, :BT], AF.Sigmoid,
                                             bias=fbias_t[:, 6 + mc:7 + mc])
                    for bl in range(2):
                        b = 2 * grp + bl
                        psl = slice(bl * NP_, (bl + 1) * NP_)
                        tsl = slice(b * S + 1, (b + 1) * S)
                        dd = stp.tile([128, 5, TT], F32, tag='st', name='fd')
                        dv = dd[:, 0:3, :NP_]
                        nc.vector.tensor_sub(dv, xn[:, :, psl].bitcast(F32), xp[:, :, psl].bitcast(F32))
                        nc.vector.tensor_mul(dv, gt[:, :, psl], dv)
                        nc.vector.tensor_add(dv, dv, xp[:, :, psl].bitcast(F32))
                        nc.vector.tensor_add(hT[:, :, tsl].bitcast(F32R), dv, pet_t[:])
                        nc.vector.tensor_copy(hT[:, :, b * S:b * S + 1].bitcast(F32R),
                                              fbias_t[:, 9:12].unsqueeze(2))

            # ========================= transformer layers ===================
            with (
                tc.tile_pool(name='w1bp', bufs=1) as w1bp,
                tc.tile_pool(name='w2bp', bufs=1) as w2bp,
                tc.tile_pool(name='w18p', bufs=2) as w18p,
                tc.tile_pool(name='w28p', bufs=2) as w28p,
                tc.tile_pool(name='wps', bufs=1) as wps,
                tc.tile_pool(name='fgp', bufs=2) as fgp,
                tc.tile_pool(name='midp', bufs=2) as midp,
                tc.tile_pool(name='h2p', bufs=4) as h2p,
                tc.tile_pool(name='clsp', bufs=2) as clsp,
                tc.tile_pool(name='amp', bufs=1) as amp,
            ):
                pending = None
                for l in range(n_layers):
                    w1_t = w1bp.tile([128, 3, 1536], BF16, tag='w1b', name='w1_t')
                    nc.sync.dma_start(w1_t[:], W1d[l])
                    w2_t = w2bp.tile([128, 12, 3, 128], BF16, tag='w2b', name='w2_t')
                    nc.sync.dma_start(w2_t[:], W2d[l])
                    w18_t = w18p.tile([128, 4, 1536], FP8, tag='w18', name='w18_t')
                    nc.sync.dma_start(w18_t[:], W18d[l])
                    w28_t = w28p.tile([128, 12, 3, 128], FP8, tag='w28', name='w28_t')
                    nc.sync.dma_start(w28_t[:], W28d[l])
                    bd_t = wps.tile([128, 3, 4, 128], F32R, tag='bd', name='bd_t')
                    nc.sync.dma_start(bd_t[:], BDd[l])
                    aw1_t = wps.tile([128, 3, 384], F32, tag='aw1', name='aw1_t')
                    nc.sync.dma_start(aw1_t[:], AW1d[l])
                    aw2_t = wps.tile([128, 3, 1024], F32, tag='aw2', name='aw2_t')
                    nc.sync.dma_start(aw2_t[:], AW2d[l])
                    bias_t = wps.tile([128, 26], F32, tag='bias', name='bias_t')
                    nc.sync.dma_start(bias_t[:], BIASd[l])
                    ab2r_t = wps.tile([1, 1024], F32, tag='ab2r', name='ab2r_t')
                    nc.sync.dma_start(ab2r_t[:], AB2Rd[l])
                    b2r_t = wps.tile([1, 3, 128], BF16, tag='b2r', name='b2r_t')
                    nc.sync.dma_start(b2r_t[:], B2Rd[l])
                    b2r8_t = wps.tile([1, 3, 128], BF16, tag='b2r8', name='b2r8_t')
                    nc.sync.dma_start(b2r8_t[:], B2R8d[l])
                    b1r_t = wps.tile([1, 12, 128], BF16, tag='b1r', name='b1r_t')
                    nc.sync.dma_start(b1r_t[:], B1Rd[l])

                    hn = hnp.tile([128, 3, NTOK], F32R, tag='hn', name='hn')
                    mh = amp.tile([128, 3, BC], F32, tag='mh', name='mh')
                    if pending is None:
                        sts = []
                        for t in range(4):
                            sl = slice(t * TT, (t + 1) * TT)
                            hs = [hT[:, c, sl] for c in range(3)]
                            sts.append((sl, hs, stats_pre(hs, TT, cgl=g['cg'][l])))
                        sqrt_gang([sts[t][2] for t in range(4)], TT)
                    else:
                        sts = pending
                    ps_u = psp.tile([128, TT], F32, tag='ps2', bufs=2, name='ps_u')
                    ps_e = psp.tile([128, TT], F32, tag='ps2', bufs=2, name='ps_e')
                    u2t = amp.tile([128, 3, BC], F32, tag='u2', name='u2t')
                    eff = amp.tile([128, 8, BC], F32, tag='eff', name='eff')
                    for t in range(4):
                        sl, hs, st = sts[t]
                        m = st[:, 0, :]
                        rsd = st[:, 1, :]
                        # LN1 apply with fused per-sample token-sum (-> mh)
                        for c in range(3):
                            tm = st[:, 2 + c, :]
                            eng = nc.gpsimd if c == 2 else nc.vector
                            eng.tensor_sub(tm, hs[c], m)
                            for j in range(2):
                                jsl = slice(j * S, (j + 1) * S)
                                nc.vector.scalar_tensor_tensor(
                                    hn[:, c, sl][:, jsl], tm[:, jsl], 1.0,
                                    rsd[:, jsl], ALU.mult, ALU.mult,
                                    accum_out=mh[:, c, 2 * t + j:2 * t + j + 1])
                        bsl = slice(2 * t, 2 * t + 2)
                        for mc in range(3):
                            for kc in range(3):
                                nc.tensor.matmul(
                                    ps_u[:, mc * BC:mc * BC + BC][:, bsl],
                                    aw1_t[:, kc, mc * 128:(mc + 1) * 128],
                                    mh[:, kc, bsl], start=(kc == 0), stop=(kc == 2))
                        if t in (1, 3):
                            hsl = slice(0, 4) if t == 1 else slice(4, 8)
                            for mc in range(3):
                                nc.scalar.activation(u2t[:, mc, hsl],
                                                     ps_u[:, mc * BC:mc * BC + BC][:, hsl],
                                                     AF.Gelu, bias=bias_t[:, mc:mc + 1])
                            for mt in range(8):
                                for kc in range(3):
                                    nc.tensor.matmul(
                                        ps_e[:, mt * BC:mt * BC + BC][:, hsl],
                                        aw2_t[:, kc, mt * 128:(mt + 1) * 128],
                                        u2t[:, kc, hsl], start=(kc == 0), stop=False)
                                nc.tensor.matmul(
                                    ps_e[:, mt * BC:mt * BC + BC][:, hsl],
                                    ab2r_t[:, mt * 128:(mt + 1) * 128],
                                    onesf_t[0:1, hsl], start=False, stop=True)
                                nc.vector.tensor_scalar(eff[:, mt, hsl],
                                                        ps_e[:, mt * BC:mt * BC + BC][:, hsl],
                                                        1.0, None, ALU.mult)  # PSUM read: DVE

                    # FFT mixer
                    KCS_F = [[0], [0, 1], [1, 2], [2]]
                    KCS_I = [[0, 1], [1, 2], [2, 3]]
                    for t in range(4):
                        sl = slice(t * TT, (t + 1) * TT)
                        fg = fgp.tile([128, 4, TT], F32R, tag='fg', name='fg')
                        for mc in range(4):
                            ps_F = psp.tile([128, TT], F32, tag='ps', name='ps_F')
                            kcs = KCS_F[mc]
                            for i, kc in enumerate(kcs):
                                nc.tensor.matmul(ps_F[:], bd_t[:, kc, mc, :], hn[:, kc, sl],
                                                 start=(i == 0), stop=(i == len(kcs) - 1))
                            for j in range(2):
                                bb = 2 * t + j
                                nc.scalar.activation(fg[:, mc, j * S:(j + 1) * S],
                                                     ps_F[:, j * S:(j + 1) * S], AF.Gelu,
                                                     scale=eff[:, mc, bb:bb + 1],
                                                     bias=eff[:, 4 + mc, bb:bb + 1])
                        for mc in range(3):
                            ps_A = psp.tile([128, TT], F32, tag='ps', name='ps_A')
                            kcs = KCS_I[mc]
                            for i, kc in enumerate(kcs):
                                nc.tensor.matmul(ps_A[:], ibd_t[:, kc, mc, :], fg[:, kc, :],
                                                 start=(i == 0), stop=(i == len(kcs) - 1))
                            nc.vector.tensor_add(hT[:, mc, sl].bitcast(F32R), hT[:, mc, sl], ps_A[:])

                    # LN2 stats (sqrt batched after all 4 tiles)
                    ln2q = []
                    for t in range(4):
                        sl = slice(t * TT, (t + 1) * TT)
                        hs = [hT[:, c, sl] for c in range(3)]
                        ln2q.append((sl, hs, stats_pre(hs, TT)))
                        if t in (1, 3):
                            sqrt_gang([ln2q[t - 1][2], ln2q[t][2]], TT)

                    # LN2 apply -> h2 fp8 (+ bf16 cls columns)
                    h2cls = clsp.tile([128, 3, BC], BF16, tag='h2c', name='h2cls')
                    h2s = []
                    for t in range(4):
                        sl, hs, st = ln2q[t]
                        m = st[:, 0, :]
                        rsd = st[:, 1, :]
                        h2 = h2p.tile([128, 4, TT], FP8, tag='h2', name='h2')
                        nc.gpsimd.memset(h2[:, 3, :], 0.0)
                        for c in range(3):
                            tm = st[:, 2 + c, :]
                            eng = nc.gpsimd if c == 2 else nc.vector
                            eng.tensor_sub(tm, hs[c], m)
                            nc.gpsimd.tensor_mul(h2[:, c, :], tm, rsd)
                            tm2 = tm.rearrange("p (j s) -> p j s", s=S)[:, :, 0]
                            rs2 = rsd.rearrange("p (j s) -> p j s", s=S)[:, :, 0]
                            nc.vector.scalar_tensor_tensor(h2cls[:, c, 2 * t:2 * t + 2],
                                                           tm2, 1.0, rs2, ALU.mult, ALU.mult)
                        h2s.append(h2)

                    # cls-column MLP in bf16 (8 columns, all samples at once)
                    midcls = clsp.tile([128, 12, BC], BF16, tag='midc', name='midcls')
                    ps_c1 = psp.tile([128, 12 * BC], F32, tag='ps', name='ps_c1')
                    for mc in range(12):
                        oc = ps_c1[:, mc * BC:(mc + 1) * BC]
                        for kc in range(3):
                            nc.tensor.matmul(oc, w1_t[:, kc, mc * 128:(mc + 1) * 128],
                                             h2cls[:, kc, :], start=(kc == 0), stop=False)
                        nc.tensor.matmul(oc, b1r_t[:, mc, :], onesb_t[0:1, :BC],
                                         start=False, stop=True)
                    nc.scalar.activation(midcls[:].rearrange("p a b -> p (a b)"), ps_c1[:],
                                         AF.Gelu)
                    ps_c2 = psp.tile([128, 3 * BC], F32, tag='ps', name='ps_c2')
                    for mc in range(3):
                        oc = ps_c2[:, mc * BC:(mc + 1) * BC]
                        for kc in range(12):
                            nc.tensor.matmul(oc, w2_t[:, kc, mc, :], midcls[:, kc, :],
                                             start=(kc == 0), stop=False)
                        nc.tensor.matmul(oc, b2r_t[:, mc, :], onesb_t[0:1, :BC],
                                         start=False, stop=True)
                    for mc in range(3):
                        ht_cls = hT[:, mc, :].rearrange("p (b s) -> p b s", s=S)[:, :, 0]
                        nc.vector.scalar_tensor_tensor(ht_cls.bitcast(F32R), ps_c2[:, mc * BC:(mc + 1) * BC],
                                                       1.0, ht_cls, ALU.mult, ALU.add)

                    # fp8 MLP (patch tokens; cls columns of the residual are
                    # skipped — the bf16 path above owns them)
                    nxt = []
                    for t in range(4):
                        mlp_tile8(ln2q[t][0], h2s[t], w18_t, w28_t, b2r8_t,
                                  onesb_t, bias_t)
                        sl, hs, _ = ln2q[t]
                        if l + 1 < n_layers:
                            nxt.append((sl, hs, stats_pre(hs, TT, cgl=g['cg'][l + 1])))
                            if t in (1, 3):
                                sqrt_gang([nxt[t - 1][2], nxt[t][2]], TT)
                    pending = nxt if l + 1 < n_layers else None

                out_ap = hT[:].rearrange("p c (b s) -> p c b s", s=S)[:, :, :, 0]
                nc.sync.dma_start(HCLS[:], out_ap)

    nc.compile()
    return nc


def _gelu_np(x):
    try:
        from scipy.special import erf
    except ImportError:
        import math
        erf = np.vectorize(math.erf)
    return x * 0.5 * (1.0 + erf(x / np.sqrt(2.0)))


def _head(hcls, g):
    x = hcls.astype(np.float64).T
    m = x.mean(1, keepdims=True)
    v = ((x - m) ** 2).mean(1, keepdims=True)
    cls = (x - m) / np.sqrt(v + EPS) * g['norm_g'] + g['norm_b']
    u = _gelu_np(cls @ g['head_w1'] + g['head_b1'])
    return ((u @ g['head_w2'])[:, 0] + g['head_b2'][0]).astype(np.float32)


def _in_maps(inputs, g):
    x = np.ascontiguousarray(inputs['x'], np.float32)
    pf = np.ascontiguousarray(inputs['patch_feats'], np.float32)
    shared = dict(
        w1=g['W1'], w2=g['W2'], w18=g['W18'], w28=g['W28'], bd=g['BD'],
        ibd=g['IBD'], aw1=g['AW1'],
        aw2=g['AW2'], bias=g['BIAS'], ab2r=g['AB2R'], b2r=g['B2R'],
        b2r8=g['B2R8'], b1r=g['B1R'],
        onesf=np.ones((1, BC), np.float32),
        onesb=_bf16(np.ones((1, TT))), pew=g['PEW'], phw=g['PHW'], gw=g['GW'],
        fbias=g['FBIAS'], pet=g['PET'],
        ones=np.ones((128, 128), np.float32),
    )
    Hp = 224 // P
    pat = x.reshape(B, 3, Hp, P, Hp, P).transpose(0, 1, 2, 4, 3, 5).reshape(B, 3, NP_, 2, 128)
    maps = []
    for i in range(NCORES):
        m = dict(shared)
        pc = pat[i * BC:(i + 1) * BC]                       # [BC,3,196,2,128]
        m['patt'] = np.ascontiguousarray(pc.transpose(4, 1, 3, 0, 2).reshape(128, 3, 2, NBP))
        m['pft'] = np.ascontiguousarray(pf[i * BC:(i + 1) * BC].reshape(NBP, 6).T)
        maps.append(m)
    return maps


def kernel(**inputs):
    inputs = {k: np.asarray(v) for k, v in inputs.items()}
    g = _prep(inputs)
    # program structure bakes per-layer ln1 gains into immediates; key on them
    key = (tuple(np.round(np.asarray(g['cg'], np.float64), 12)),)
    if _CACHE.get('key') != key:
        _CACHE['prog'] = _build(g)
        _CACHE['key'] = key
    nc = _CACHE['prog']
    res = run_bass_kernel_spmd(nc, _in_maps(inputs, g), list(range(NCORES)))
    _CACHE['last_res'] = res
    _CACHE['last_g'] = g
    hcls = np.concatenate(
        [r['hcls'].transpose(1, 0, 2).reshape(D, BC) for r in res.results], axis=1)
    return _head(hcls, g)


if __name__ == '__main__':
    d = np.load('/root/problem/ref_data.npz')
    inputs = {k: d[k] for k in d.files if k != 'expected'}
    y = kernel(**inputs)
    exp = d['expected']
    err = np.abs(y - exp)
    print("max abs err:", err.max())
    print("Relative error:", err.max() / np.abs(exp).max())


# revision 49
# speedup vs baseline: 1.0852x; 1.0671x over previous
"""Trainium2 Bass kernel for nn_FFTPermeabilityPredictorPatchPhysics.

Sharding: pure data parallel — 8 samples per NeuronCore, weights replicated.
On-device layout: residual stream transposed, hT [3x128 d-chunks, 1576 tok],
kept in SBUF for all 12 layers. FFT/iFFT as block-diagonal matmuls over a
512-row padded frequency layout (head h -> rows 64h+32s+f). LN stats via
ones-matmul partition reductions broadcast to all partitions; the adaptive
spectral filter is fused into the ACT-engine gelu via per-partition
scale/bias. The MLP runs fp8e4 DoubleRow (K=256 per instruction) for the
196 patch tokens with weight scale 64 folded into the gelu scale and the
residual scalar_tensor_tensor; the cls token column (which feeds the head
directly, without the 1/197 mean dilution of patch tokens) is recomputed
in bf16 against the same-layer bf16 weights. All weight folding done
host-side in numpy: double-LN collapse, pre_g/ln2_g into following
matmuls, base_filter and (1+ap) into amlp_w2, 1/197 token-mean into
amlp_w1, DFT matrices baked. Final LN + head on the 64 cls vectors runs
host-side in float64.
"""
import numpy as np

import concourse.bacc as bacc
import concourse.mybir as mybir
import concourse.tile as tile
from concourse.bass_utils import run_bass_kernel_spmd

F32 = mybir.dt.float32
F32R = mybir.dt.float32r
BF16 = mybir.dt.bfloat16
FP8 = mybir.dt.float8e4
AF = mybir.ActivationFunctionType
ALU = mybir.AluOpType
DR = mybir.MatmulPerfMode.DoubleRow

B, D, H, HD, FB, S, L, P, NP_ = 64, 384, 8, 48, 25, 197, 12, 16, 196
EPS = 1e-5
FR = 512
NCORES = 8
BC = B // NCORES     # 8 samples/core
NTOK = BC * S        # 1576
TT = 394             # token tile = 2 samples
NBP = BC * NP_       # 1568
BT = 392             # patch tile = 2 samples
WS = 64.0            # fp8 weight scale for both MLP matmuls
IWS = 1.0 / WS

_CACHE = {}


def _build_dft():
    n = np.arange(HD)
    k = np.arange(FB)
    ang = -2 * np.pi * np.outer(n, k) / HD
    Cr = np.cos(ang) / np.sqrt(HD)
    Ci = np.sin(ang) / np.sqrt(HD)
    A = np.zeros((FB, HD))
    Bm = np.zeros((FB, HD))
    ifft_w = np.exp(2j * np.pi * np.outer(np.arange(HD), np.arange(HD)) / HD) / np.sqrt(HD)
    for j in range(FB):
        fr = np.zeros(HD, complex)
        fi = np.zeros(HD, complex)
        fr[j] = 1.0
        fi[j] = 1.0j
        if 0 < j < HD - FB + 1:
            fr[HD - j] = 1.0
            fi[HD - j] = -1.0j
        A[j] = (ifft_w @ fr).real
        Bm[j] = (ifft_w @ fi).real
    return Cr, Ci, A, Bm


def _prep(inp, n_layers=L):
    f = {k: np.asarray(v, np.float64) for k, v in inp.items()}
    Cr, Ci, A, Bm = _build_dft()

    BDb = np.zeros((D, FR))
    iBD = np.zeros((FR, D))
    for h in range(H):
        BDb[48 * h:48 * h + 48, 64 * h:64 * h + FB] = Cr
        BDb[48 * h:48 * h + 48, 64 * h + 32:64 * h + 32 + FB] = Ci
        iBD[64 * h:64 * h + FB, 48 * h:48 * h + 48] = A
        iBD[64 * h + 32:64 * h + 32 + FB, 48 * h:48 * h + 48] = Bm

    cg = f['ln1_g'].mean(1)
    assert np.abs(f['ln1_g'] - cg[:, None]).max() < 1e-12, "ln1_g must be constant/layer"
    assert np.abs(f['ln1_b'] - f['ln1_b'].mean(1)[:, None]).max() < 1e-12
    assert np.allclose(f['pe_ln_g'], 1.0) and np.allclose(f['pe_ln_b'], 0.0), "pe_ln fold"

    BD_l = np.einsum('ld,df->ldf', cg[:, None] * f['pre_g'], BDb)
    bdbias_l = np.einsum('ld,df->lf', f['pre_b'], BDb)

    aw1p = np.einsum('ld,lde->lde', cg[:, None] * f['pre_g'], f['amlp_w1']) / S
    ab1p = np.einsum('ld,lde->le', f['pre_b'], f['amlp_w1']) + f['amlp_b1']

    aw2pp = np.zeros((L, D, 2 * FR))
    ab2pp = np.zeros((L, 2 * FR))
    aw2, ab2 = f['amlp_w2'], f['amlp_b2']
    bf, bb = f['base_filter'], f['base_bias']
    for h in range(H):
        for s in range(2):
            for fq in range(FB):
                r = 64 * h + 32 * s + fq
                c0 = h * (FB * 2) + fq * 2
                wf = bf[:, h, fq][:, None] * aw2[:, :, c0]
                bf_ = bf[:, h, fq] * ab2[:, c0] + bf[:, h, fq]
                aw2pp[:, :, r] = wf
                ab2pp[:, r] = bf_
                aw2pp[:, :, FR + r] = bdbias_l[:, r][:, None] * wf
                ab2pp[:, FR + r] = bdbias_l[:, r] * bf_
                if s == 0:
                    aw2pp[:, :, FR + r] += aw2[:, :, c0 + 1]
                    ab2pp[:, FR + r] += bb[:, h, fq] + ab2[:, c0 + 1]

    w1p = np.einsum('ld,lde->lde', f['ln2_g'], f['mlp_w1'])
    b1p = np.einsum('ld,lde->le', f['ln2_b'], f['mlp_w1']) + f['mlp_b1']

    a32 = lambda x: np.ascontiguousarray(x, np.float32)
    g = {}
    g['cg'] = cg
    g['W1'] = _bf16(w1p.reshape(L, 3, 128, 4 * D).transpose(0, 2, 1, 3))            # [L,128,3,1536] bf16
    g['W2'] = _bf16(f['mlp_w2'].reshape(L, 12, 128, 3, 128).transpose(0, 2, 1, 3, 4))
    # fp8 copies (scaled by WS); W1 padded to 4 k-chunks for DoubleRow pairs
    w18 = np.zeros((L, 128, 4, 4 * D))
    w18[:, :, :3, :] = WS * w1p.reshape(L, 3, 128, 4 * D).transpose(0, 2, 1, 3)
    g['W18'] = _fp8(w18)                                                            # [L,128,4,1536]
    g['W28'] = _fp8(WS * f['mlp_w2'].reshape(L, 12, 128, 3, 128).transpose(0, 2, 1, 3, 4))
    g['BD'] = a32(BD_l.reshape(L, 3, 128, 4, 128).transpose(0, 2, 1, 3, 4))
    g['IBD'] = a32(iBD.reshape(4, 128, 3, 128).transpose(1, 0, 2, 3))
    g['AW1'] = a32(aw1p.reshape(L, 3, 128, D).transpose(0, 2, 1, 3))
    g['AB2R'] = a32(ab2pp[:, None, :])                                              # [L,1,1024]
    g['B2R'] = _bf16(f['mlp_b2'][:, None, :].reshape(L, 1, 3, 128))
    g['B2R8'] = _bf16(WS * f['mlp_b2'][:, None, :].reshape(L, 1, 3, 128))
    g['B1R'] = _bf16(b1p[:, None, :].reshape(L, 1, 12, 128))
    g['AW2'] = a32(aw2pp.reshape(L, 3, 128, 2 * FR).transpose(0, 2, 1, 3))
    # packed per-layer biases [L,128,26]: 0-2 ab1, 3-10 ab2, 11-22 b1, 23-25 b2
    bias = np.zeros((L, 128, 26))
    bias[:, :, 0:3] = ab1p.reshape(L, 3, 128).transpose(0, 2, 1)
    bias[:, :, 3:11] = ab2pp.reshape(L, 8, 128).transpose(0, 2, 1)
    bias[:, :, 11:23] = b1p.reshape(L, 12, 128).transpose(0, 2, 1)
    bias[:, :, 23:26] = f['mlp_b2'].reshape(L, 3, 128).transpose(0, 2, 1)
    g['BIAS'] = a32(bias)
    g['PEW'] = a32(f['pe_w'].reshape(3, 2, 128, 128).transpose(2, 0, 1, 3))          # [128,3,2,128]
    g['PHW'] = a32(f['phys_w'].reshape(6, 3, 128))                                   # [6,3,128]
    g['GW'] = a32(f['gate_w'].reshape(6, 128, 3, 128).transpose(1, 0, 2, 3))         # [128,6,3,128]
    fbias = np.zeros((128, 12))  # 0-2 peb, 3-5 phb, 6-8 gb, 9-11 clspe
    fbias[:, 0:3] = f['pe_b'].T
    fbias[:, 3:6] = f['phys_b'].reshape(3, 128).T
    fbias[:, 6:9] = f['gate_b'].reshape(3, 128).T
    fbias[:, 9:12] = (f['cls_token'][0, 0] + f['pos_embed'][0, 0]).reshape(3, 128).T
    g['FBIAS'] = a32(fbias)
    g['PET'] = a32(f['pos_embed'][0, 1:].T.reshape(3, 128, NP_).transpose(1, 0, 2))  # [128,3,196]
    for kk in ('norm_g', 'norm_b', 'head_w1', 'head_b1', 'head_w2', 'head_b2'):
        g[kk] = f[kk]
    g['n_layers'] = n_layers
    return g


def _bf16(x):
    import ml_dtypes
    return np.ascontiguousarray(np.asarray(x, np.float32), dtype=ml_dtypes.bfloat16)


def _fp8(x):
    import ml_dtypes
    return np.ascontiguousarray(np.asarray(x, np.float32), dtype=ml_dtypes.float8_e4m3)


def _build(g):
    import math
    n_layers = g['n_layers']
    nc = bacc.Bacc('TRN2', target_bir_lowering=False, debug=False)
    for val in (EPS,):
        t = nc.alloc_sbuf_tensor(f"const-f32-{val}", [128, 1], F32)
        nc.gpsimd.memset(t.ap(), val)
        nc.const_aps.aps[(F32, val)] = t.ap()
    nc.all_engine_barrier()

    di = lambda name, shape, dt: nc.dram_tensor(name, list(shape), dt, kind="ExternalInput")
    PATd = di('patt', (128, 3, 2, NBP), F32R)
    PFT = di('pft', (6, NBP), F32R)
    W1d = di('w1', (L, 128, 3, 1536), BF16)
    W2d = di('w2', (L, 128, 12, 3, 128), BF16)
    W18d = di('w18', (L, 128, 4, 1536), FP8)
    W28d = di('w28', (L, 128, 12, 3, 128), FP8)
    BDd = di('bd', (L, 128, 3, 4, 128), F32R)
    IBDd = di('ibd', (128, 4, 3, 128), F32R)
    AW1d = di('aw1', (L, 128, 3, 384), F32)
    AW2d = di('aw2', (L, 128, 3, 1024), F32)
    BIASd = di('bias', (L, 128, 26), F32)
    AB2Rd = di('ab2r', (L, 1, 1024), F32)
    B2Rd = di('b2r', (L, 1, 3, 128), BF16)
    B2R8d = di('b2r8', (L, 1, 3, 128), BF16)
    B1Rd = di('b1r', (L, 1, 12, 128), BF16)
    ONFd = di('onesf', (1, BC), F32)
    ONBd = di('onesb', (1, TT), BF16)
    PEWd = di('pew', (128, 3, 2, 128), F32R)
    PHWd = di('phw', (6, 3, 128), F32R)
    GWd = di('gw', (128, 6, 3, 128), F32R)
    FBIASd = di('fbias', (128, 12), F32)
    PETd = di('pet', (128, 3, NP_), F32)
    ONESd = di('ones', (128, 128), F32R)
    HCLS = nc.dram_tensor('hcls', [128, 3, BC], F32, kind="ExternalOutput")

    with tile.TileContext(nc) as tc:
        with (
            tc.tile_pool(name='const', bufs=1) as cp,
            tc.tile_pool(name='persist', bufs=1) as pp,
            tc.tile_pool(name='hnp', bufs=1) as hnp,
            tc.tile_pool(name='xqp', bufs=4) as xqp,
            tc.tile_pool(name='stp', bufs=4) as stp,
            tc.tile_pool(name='psp', bufs=6, space='PSUM') as psp,
        ):
            ones_t = cp.tile([128, 128], F32R, name='ones_t')
            nc.sync.dma_start(ones_t[:], ONESd[:])
            ibd_t = cp.tile([128, 4, 3, 128], F32R, name='ibd_t')
            nc.sync.dma_start(ibd_t[:], IBDd[:])
            onesf_t = cp.tile([1, BC], F32, name='onesf_t')
            nc.sync.dma_start(onesf_t[:], ONFd[:])
            onesb_t = cp.tile([1, TT], BF16, name='onesb_t')
            nc.sync.dma_start(onesb_t[:], ONBd[:])
            fbias_t = cp.tile([128, 12], F32, name='fbias_t')
            nc.sync.dma_start(fbias_t[:], FBIASd[:])
            pet_t = cp.tile([128, 3, NP_], F32, name='pet_t')
            nc.sync.dma_start(pet_t[:], PETd[:])

            hT = pp.tile([128, 3, NTOK], F32, name='hT')

            def stats_pre(srcs, tlen, cgl=None, pstag='ps', on_act=False):
                """LN stats (up to 1/ve) for one token tile; srcs = 3
                [128,tlen] f32 APs. Double-LN folds to a single rsqrt:
                rs1*rs2 = rsqrt((cg^2+eps)*v + eps^2). Act-table-free:
                the Sqrt is emitted separately by stats_sqrt."""
                xq = xqp.tile([128, 3, TT], F32R, tag='xq', name='xq')
                for c in range(3):
                    eng = nc.vector if c == 0 else nc.gpsimd
                    eng.tensor_mul(xq[:, c, :tlen], srcs[c], srcs[c])
                ps_s = psp.tile([128, TT], F32, tag='ps2', bufs=2, name='ps_s')
                ps_q = psp.tile([128, TT], F32, tag='ps2', bufs=2, name='ps_q')
                for c in range(3):
                    nc.tensor.matmul(ps_s[:, :tlen], ones_t[:], srcs[c].bitcast(F32R),
                                     start=(c == 0), stop=(c == 2))
                for c in range(3):
                    nc.tensor.matmul(ps_q[:, :tlen], ones_t[:], xq[:, c, :tlen],
                                     start=(c == 0), stop=(c == 2))
                if cgl is None:
                    A, Bc_ = 1.0, EPS
                else:
                    A = float(cgl) * float(cgl) + EPS
                    Bc_ = EPS * EPS
                st = stp.tile([128, 5, TT], F32, tag='st', name='st')
                m = st[:, 0, :tlen]
                rsd = st[:, 1, :tlen]
                mm = st[:, 2, :tlen]
                t1 = st[:, 3, :tlen]
                ve = st[:, 4, :tlen]
                u = st[:, 3, :tlen]  # t1's row; t1 is dead once ve is formed
                # PSUM readers must be DVE/Act (GPSIMD cannot access PSUM).
                # on_act runs the PSUM-consuming stats on the Act engine
                # (Copy/Square live in every act table): used for the
                # next-layer stats at the tail of a layer, where Act idles
                # and early PSUM reads unblock the next layer's matmuls.
                if on_act:
                    nc.scalar.activation(m, ps_s[:, :tlen], AF.Copy, scale=1.0 / D)
                    nc.scalar.activation(t1, ps_q[:, :tlen], AF.Copy, bias=Bc_,
                                         scale=A / D)
                    nc.scalar.activation(mm, ps_s[:, :tlen], AF.Square,
                                         scale=math.sqrt(A) / D)
                    nc.vector.scalar_tensor_tensor(ve, mm, -1.0, t1, ALU.mult, ALU.add)
                else:
                    # B (eps^2 for the folded double-LN, eps for LN2) is
                    # negligible vs ve ~ A*var = O(1): drop it and save an op.
                    nc.vector.tensor_scalar(m, ps_s[:, :tlen], 1.0 / D, None, ALU.mult)
                    nc.vector.scalar_tensor_tensor(mm, m, A, m, ALU.mult, ALU.mult)
                    nc.vector.scalar_tensor_tensor(ve, ps_q[:, :tlen], A / D, mm,
                                                   ALU.mult, ALU.subtract)
                nc.vector.reciprocal(u, ve)
                return st

            def stats_sqrt(st, tlen, gate=None):
                if gate is None:
                    nc.scalar.activation(st[:, 1, :tlen], st[:, 3, :tlen], AF.Sqrt)
                else:
                    nc.scalar.activation(st[:, 1, :tlen], st[:, 3, :tlen], AF.Sqrt,
                                         scale=gate)

            def sqrt_gang(sts_list, tlen):
                """Emit the sqrts of a stats batch gated on the LAST tile's
                recip, so the Act-engine scheduler runs them back-to-back
                (one sqrt<->gelu table swap per batch instead of one per
                tile)."""
                gate = stp.tile([128, 1], F32, tag='gate', name='gate', bufs=4)
                last_u = sts_list[-1][:, 3, 0:1]
                nc.vector.tensor_scalar(gate, last_u, 0.0, 1.0, ALU.mult, ALU.add)
                for st in sts_list:
                    stats_sqrt(st, tlen, gate=gate)

            def mlp_tile8(sl, h2, w18_t, w28_t, b2r8_t, onesb_t, bias_t):
                """fp8 DoubleRow MLP for one 2-sample token tile; the two cls
                columns of the residual are left to the bf16 cls path."""
                mid = midp.tile([128, 12, TT], FP8, tag='mid', name='mid')
                for grp in range(3):
                    pss = []
                    for mci in range(4):
                        mc = grp * 4 + mci
                        ps_m = psp.tile([128, TT], F32, tag='ps', name='ps_m')
                        for j in range(2):
                            nc.tensor.matmul(
                                ps_m[:], w18_t[:, 2 * j:2 * j + 2, mc * 128:(mc + 1) * 128],
                                h2[:, 2 * j:2 * j + 2, :], start=(j == 0), stop=(j == 1),
                                perf_mode=DR)
                        pss.append((mc, ps_m))
                    for mc, ps_m in pss:
                        nc.scalar.activation(mid[:, mc, :], ps_m[:], AF.Gelu,
                                             scale=IWS, bias=bias_t[:, 11 + mc:12 + mc])
                for mc in range(3):
                    ps_o = psp.tile([128, TT], F32, tag='ps', name='ps_o')
                    for j in range(6):
                        nc.tensor.matmul(ps_o[:], w28_t[:, 2 * j:2 * j + 2, mc, :],
                                         mid[:, 2 * j:2 * j + 2, :],
                                         start=(j == 0), stop=False, perf_mode=DR)
                    nc.tensor.matmul(ps_o[:], b2r8_t[:, mc, :], onesb_t[0:1, :TT],
                                     start=False, stop=True)
                    nc.vector.scalar_tensor_tensor(
                        hT[:, mc, sl].bitcast(F32R), ps_o[:], IWS,
                        hT[:, mc, sl], ALU.mult, ALU.add)
                    pcls = ps_o.rearrange("p (j s) -> p j s", s=S)[:, :, 0]
                    htc = hT[:, mc, sl].rearrange("p (j s) -> p j s", s=S)[:, :, 0]
                    nc.vector.scalar_tensor_tensor(
                        htc.bitcast(F32R), pcls, -IWS, htc, ALU.mult, ALU.add)

            # ================= front (streamed per 2-sample group) ==========
            with (
                tc.tile_pool(name='fgrp', bufs=2) as fg_,
                tc.tile_pool(name='fw', bufs=1) as fw,
            ):
                pft_t = fw.tile([6, NBP], F32R, name='pft_t')
                nc.sync.dma_start(pft_t[:], PFT[:])
                pew_t = fw.tile([128, 3, 2, 128], F32R, name='pew_t')
                nc.sync.dma_start(pew_t[:], PEWd[:])
                phw_t = fw.tile([6, 3, 128], F32R, name='phw_t')
                nc.sync.dma_start(phw_t[:], PHWd[:])
                for grp in range(4):
                    sl = slice(grp * BT, (grp + 1) * BT)
                    patg = fg_.tile([128, 3, 2, BT], F32R, tag='patg', name='patg')
                    for c in range(3):
                        nc.sync.dma_start(patg[:, c], PATd[:, c, :, sl])
                    ximg = fg_.tile([128, 3, BT], F32R, tag='ximg', name='ximg')
                    xn = fg_.tile([128, 3, BT], F32R, tag='xn', name='xn')
                    xp = fg_.tile([128, 3, BT], F32R, tag='xp', name='xp')
                    gt = fg_.tile([128, 3, BT], F32, tag='gt', name='gt')
                    for c in range(3):
                        ps_pe = psp.tile([128, TT], F32, tag='ps', name='ps_pe')
                        for kc in range(2):
                            nc.tensor.matmul(ps_pe[:, :BT], pew_t[:, c, kc, :], patg[:, c, kc, :],
                                             start=(kc == 0), stop=(kc == 1))
                        nc.scalar.activation(ximg[:, c, :], ps_pe[:, :BT], AF.Identity,
                                             bias=fbias_t[:, c:c + 1])
                    if grp == 0:
                        gw_t = fw.tile([128, 6, 3, 128], F32R, name='gw_t')
                        nc.sync.dma_start(gw_t[:], GWd[:])
                    xi = [ximg[:, c, :].bitcast(F32) for c in range(3)]
                    st = stats_pre(xi, BT)
                    stats_sqrt(st, BT)
                    m = st[:, 0, :BT]
                    rsd = st[:, 1, :BT]
                    for c in range(3):
                        eng = nc.gpsimd if c == 2 else nc.vector
                        tm = st[:, 2 + c, :BT]
                        eng.tensor_sub(tm, xi[c], m)
                        eng.tensor_mul(xn[:, c, :], tm, rsd)
                    for mc in range(3):
                        ps_ph = psp.tile([128, TT], F32, tag='ps', name='ps_ph')
                        nc.tensor.matmul(ps_ph[:, :BT], phw_t[:, mc, :], pft_t[:, sl],
                                         start=True, stop=True)
                        nc.scalar.activation(xp[:, mc, :], ps_ph[:, :BT], AF.Identity,
                                             bias=fbias_t[:, 3 + mc:4 + mc])
                    for mc in range(3):
                        ps_g = psp.tile([128, TT], F32, tag='ps', name='ps_g')
                        for kc in range(6):
                            rhs = xn[:, kc, :] if kc < 3 else xp[:, kc - 3, :]
                            nc.tensor.matmul(ps_g[:, :BT], gw_t[:, kc, mc, :], rhs,
                                             start=(kc == 0), stop=(kc == 5))
                        nc.scalar.activation(gt[:, mc, :], ps_g[:, :BT], AF.Sigmoid,
                                             bias=fbias_t[:, 6 + mc:7 + mc])
                    for bl in range(2):
                        b = 2 * grp + bl
                        psl = slice(bl * NP_, (bl + 1) * NP_)
                        tsl = slice(b * S + 1, (b + 1) * S)
                        dd = stp.tile([128, 5, TT], F32, tag='st', name='fd')
                        dv = dd[:, 0:3, :NP_]
                        nc.vector.tensor_sub(dv, xn[:, :, psl].bitcast(F32), xp[:, :, psl].bitcast(F32))
                        nc.vector.tensor_mul(dv, gt[:, :, psl], dv)
                        nc.vector.tensor_add(dv, dv, xp[:, :, psl].bitcast(F32))
                        nc.vector.tensor_add(hT[:, :, tsl].bitcast(F32R), dv, pet_t[:])
                        nc.vector.tensor_copy(hT[:, :, b * S:b * S + 1].bitcast(F32R),
                                              fbias_t[:, 9:12].unsqueeze(2))

            # ========================= transformer layers ===================
            with (
                tc.tile_pool(name='w1bp', bufs=1) as w1bp,
                tc.tile_pool(name='w2bp', bufs=1) as w2bp,
                tc.tile_pool(name='w18p', bufs=2) as w18p,
                tc.tile_pool(name='w28p', bufs=2) as w28p,
                tc.tile_pool(name='wps', bufs=1) as wps,
                tc.tile_pool(name='fgp', bufs=2) as fgp,
                tc.tile_pool(name='midp', bufs=2) as midp,
                tc.tile_pool(name='h2p', bufs=4) as h2p,
                tc.tile_pool(name='clsp', bufs=2) as clsp,
                tc.tile_pool(name='amp', bufs=1) as amp,
            ):
                pending = None
                for l in range(n_layers):
                    w1_t = w1bp.tile([128, 3, 1536], BF16, tag='w1b', name='w1_t')
                    nc.sync.dma_start(w1_t[:], W1d[l])
                    w2_t = w2bp.tile([128, 12, 3, 128], BF16, tag='w2b', name='w2_t')
                    nc.sync.dma_start(w2_t[:], W2d[l])
                    w18_t = w18p.tile([128, 4, 1536], FP8, tag='w18', name='w18_t')
                    nc.sync.dma_start(w18_t[:], W18d[l])
                    w28_t = w28p.tile([128, 12, 3, 128], FP8, tag='w28', name='w28_t')
                    nc.sync.dma_start(w28_t[:], W28d[l])
                    bd_t = wps.tile([128, 3, 4, 128], F32R, tag='bd', name='bd_t')
                    nc.sync.dma_start(bd_t[:], BDd[l])
                    aw1_t = wps.tile([128, 3, 384], F32, tag='aw1', name='aw1_t')
                    nc.sync.dma_start(aw1_t[:], AW1d[l])
                    aw2_t = wps.tile([128, 3, 1024], F32, tag='aw2', name='aw2_t')
                    nc.sync.dma_start(aw2_t[:], AW2d[l])
                    bias_t = wps.tile([128, 26], F32, tag='bias', name='bias_t')
                    nc.sync.dma_start(bias_t[:], BIASd[l])
                    ab2r_t = wps.tile([1, 1024], F32, tag='ab2r', name='ab2r_t')
                    nc.sync.dma_start(ab2r_t[:], AB2Rd[l])
                    b2r_t = wps.tile([1, 3, 128], BF16, tag='b2r', name='b2r_t')
                    nc.sync.dma_start(b2r_t[:], B2Rd[l])
                    b2r8_t = wps.tile([1, 3, 128], BF16, tag='b2r8', name='b2r8_t')
                    nc.sync.dma_start(b2r8_t[:], B2R8d[l])
                    b1r_t = wps.tile([1, 12, 128], BF16, tag='b1r', name='b1r_t')
                    nc.sync.dma_start(b1r_t[:], B1Rd[l])

                    hn = hnp.tile([128, 3, NTOK], F32R, tag='hn', name='hn')
                    mh = amp.tile([128, 3, BC], F32, tag='mh', name='mh')
                    if pending is None:
                        sts = []
                        for t in range(4):
                            sl = slice(t * TT, (t + 1) * TT)
                            hs = [hT[:, c, sl] for c in range(3)]
                            sts.append((sl, hs, stats_pre(hs, TT, cgl=g['cg'][l])))
                        sqrt_gang([sts[t][2] for t in range(4)], TT)
                    else:
                        sts = pending
                    ps_u = psp.tile([128, TT], F32, tag='ps2', bufs=2, name='ps_u')
                    ps_e = psp.tile([128, TT], F32, tag='ps2', bufs=2, name='ps_e')
                    u2t = amp.tile([128, 3, BC], F32, tag='u2', name='u2t')
                    eff = amp.tile([128, 8, BC], F32, tag='eff', name='eff')
                    for t in range(4):
                        sl, hs, st = sts[t]
                        m = st[:, 0, :]
                        rsd = st[:, 1, :]
                        # LN1 apply with fused per-sample token-sum (-> mh)
                        for c in range(3):
                            tm = st[:, 2 + c, :]
                            eng = nc.gpsimd if c == 2 else nc.vector
                            eng.tensor_sub(tm, hs[c], m)
                            for j in range(2):
                                jsl = slice(j * S, (j + 1) * S)
                                nc.vector.scalar_tensor_tensor(
                                    hn[:, c, sl][:, jsl], tm[:, jsl], 1.0,
                                    rsd[:, jsl], ALU.mult, ALU.mult,
                                    accum_out=mh[:, c, 2 * t + j:2 * t + j + 1])
                        bsl = slice(2 * t, 2 * t + 2)
                        for mc in range(3):
                            for kc in range(3):
                                nc.tensor.matmul(
                                    ps_u[:, mc * BC:mc * BC + BC][:, bsl],
                                    aw1_t[:, kc, mc * 128:(mc + 1) * 128],
                                    mh[:, kc, bsl], start=(kc == 0), stop=(kc == 2))
                        if t in (1, 3):
                            hsl = slice(0, 4) if t == 1 else slice(4, 8)
                            for mc in range(3):
                                nc.scalar.activation(u2t[:, mc, hsl],
                                                     ps_u[:, mc * BC:mc * BC + BC][:, hsl],
                                                     AF.Gelu, bias=bias_t[:, mc:mc + 1])
                            for mt in range(8):
                                for kc in range(3):
                                    nc.tensor.matmul(
                                        ps_e[:, mt * BC:mt * BC + BC][:, hsl],
                                        aw2_t[:, kc, mt * 128:(mt + 1) * 128],
                                        u2t[:, kc, hsl], start=(kc == 0), stop=False)
                                nc.tensor.matmul(
                                    ps_e[:, mt * BC:mt * BC + BC][:, hsl],
                                    ab2r_t[:, mt * 128:(mt + 1) * 128],
                                    onesf_t[0:1, hsl], start=False, stop=True)
                                nc.vector.tensor_scalar(eff[:, mt, hsl],
                                                        ps_e[:, mt * BC:mt * BC + BC][:, hsl],
                                                        1.0, None, ALU.mult)  # PSUM read: DVE

                    # FFT mixer
                    KCS_F = [[0], [0, 1], [1, 2], [2]]
                    KCS_I = [[0, 1], [1, 2], [2, 3]]
                    for t in range(4):
                        sl = slice(t * TT, (t + 1) * TT)
                        fg = fgp.tile([128, 4, TT], F32R, tag='fg', name='fg')
                        for mc in range(4):
                            ps_F = psp.tile([128, TT], F32, tag='ps', name='ps_F')
                            kcs = KCS_F[mc]
                            for i, kc in enumerate(kcs):
                                nc.tensor.matmul(ps_F[:], bd_t[:, kc, mc, :], hn[:, kc, sl],
                                                 start=(i == 0), stop=(i == len(kcs) - 1))
                            for j in range(2):
                                bb = 2 * t + j
                                nc.scalar.activation(fg[:, mc, j * S:(j + 1) * S],
                                                     ps_F[:, j * S:(j + 1) * S], AF.Gelu,
                                                     scale=eff[:, mc, bb:bb + 1],
                                                     bias=eff[:, 4 + mc, bb:bb + 1])
                        for mc in range(3):
                            ps_A = psp.tile([128, TT], F32, tag='ps', name='ps_A')
                            kcs = KCS_I[mc]
                            for i, kc in enumerate(kcs):
                                nc.tensor.matmul(ps_A[:], ibd_t[:, kc, mc, :], fg[:, kc, :],
                                                 start=(i == 0), stop=(i == len(kcs) - 1))
                            nc.vector.tensor_add(hT[:, mc, sl].bitcast(F32R), hT[:, mc, sl], ps_A[:])

                    # LN2 stats (sqrt batched after all 4 tiles)
                    ln2q = []
                    for t in range(4):
                        sl = slice(t * TT, (t + 1) * TT)
                        hs = [hT[:, c, sl] for c in range(3)]
                        ln2q.append((sl, hs, stats_pre(hs, TT)))
                        if t in (1, 3):
                            sqrt_gang([ln2q[t - 1][2], ln2q[t][2]], TT)

                    # LN2 apply -> h2 fp8 (+ bf16 cls columns)
                    h2cls = clsp.tile([128, 3, BC], BF16, tag='h2c', name='h2cls')
                    h2s = []
                    for t in range(4):
                        sl, hs, st = ln2q[t]
                        m = st[:, 0, :]
                        rsd = st[:, 1, :]
                        h2 = h2p.tile([128, 4, TT], FP8, tag='h2', name='h2')
                        nc.gpsimd.memset(h2[:, 3, :], 0.0)
                        for c in range(3):
                            tm = st[:, 2 + c, :]
                            eng = nc.gpsimd if c == 2 else nc.vector
                            eng.tensor_sub(tm, hs[c], m)
                            nc.gpsimd.tensor_mul(h2[:, c, :], tm, rsd)
                            tm2 = tm.rearrange("p (j s) -> p j s", s=S)[:, :, 0]
                            rs2 = rsd.rearrange("p (j s) -> p j s", s=S)[:, :, 0]
                            nc.vector.scalar_tensor_tensor(h2cls[:, c, 2 * t:2 * t + 2],
                                                           tm2, 1.0, rs2, ALU.mult, ALU.mult)
                        h2s.append(h2)

                    # cls-column MLP in bf16 (8 columns, all samples at once)
                    midcls = clsp.tile([128, 12, BC], BF16, tag='midc', name='midcls')
                    ps_c1 = psp.tile([128, 12 * BC], F32, tag='ps', name='ps_c1')
                    for mc in range(12):
                        oc = ps_c1[:, mc * BC:(mc + 1) * BC]
                        for kc in range(3):
                            nc.tensor.matmul(oc, w1_t[:, kc, mc * 128:(mc + 1) * 128],
                                             h2cls[:, kc, :], start=(kc == 0), stop=False)
                        nc.tensor.matmul(oc, b1r_t[:, mc, :], onesb_t[0:1, :BC],
                                         start=False, stop=True)
                    nc.scalar.activation(midcls[:].rearrange("p a b -> p (a b)"), ps_c1[:],
                                         AF.Gelu)
                    ps_c2 = psp.tile([128, 3 * BC], F32, tag='ps', name='ps_c2')
                    for mc in range(3):
                        oc = ps_c2[:, mc * BC:(mc + 1) * BC]
                        for kc in range(12):
                            nc.tensor.matmul(oc, w2_t[:, kc, mc, :], midcls[:, kc, :],
                                             start=(kc == 0), stop=False)
                        nc.tensor.matmul(oc, b2r_t[:, mc, :], onesb_t[0:1, :BC],
                                         start=False, stop=True)
                    for mc in range(3):
                        ht_cls = hT[:, mc, :].rearrange("p (b s) -> p b s", s=S)[:, :, 0]
                        nc.vector.scalar_tensor_tensor(ht_cls.bitcast(F32R), ps_c2[:, mc * BC:(mc + 1) * BC],
                                                       1.0, ht_cls, ALU.mult, ALU.add)

                    # fp8 MLP (patch tokens; cls columns of the residual are
                    # skipped — the bf16 path above owns them)
                    nxt = []
                    for t in range(4):
                        mlp_tile8(ln2q[t][0], h2s[t], w18_t, w28_t, b2r8_t,
                                  onesb_t, bias_t)
                        sl, hs, _ = ln2q[t]
                        if l + 1 < n_layers:
                            nxt.append((sl, hs, stats_pre(hs, TT, cgl=g['cg'][l + 1])))
                            if t in (1, 3):
                                sqrt_gang([nxt[t - 1][2], nxt[t][2]], TT)
                    pending = nxt if l + 1 < n_layers else None

                out_ap = hT[:].rearrange("p c (b s) -> p c b s", s=S)[:, :, :, 0]
                nc.sync.dma_start(HCLS[:], out_ap)

    nc.compile()
    return nc


def _gelu_np(x):
    try:
        from scipy.special import erf
    except ImportError:
        import math
        erf = np.vectorize(math.erf)
    return x * 0.5 * (1.0 + erf(x / np.sqrt(2.0)))


def _head(hcls, g):
    x = hcls.astype(np.float64).T
    m = x.mean(1, keepdims=True)
    v = ((x - m) ** 2).mean(1, keepdims=True)
    cls = (x - m) / np.sqrt(v + EPS) * g['norm_g'] + g['norm_b']
    u = _gelu_np(cls @ g['head_w1'] + g['head_b1'])
    return ((u @ g['head_w2'])[:, 0] + g['head_b2'][0]).astype(np.float32)


def _in_maps(inputs, g):
    x = np.ascontiguousarray(inputs['x'], np.float32)
    pf = np.ascontiguousarray(inputs['patch_feats'], np.float32)
    shared = dict(
        w1=g['W1'], w2=g['W2'], w18=g['W18'], w28=g['W28'], bd=g['BD'],
        ibd=g['IBD'], aw1=g['AW1'],
        aw2=g['AW2'], bias=g['BIAS'], ab2r=g['AB2R'], b2r=g['B2R'],
        b2r8=g['B2R8'], b1r=g['B1R'],
        onesf=np.ones((1, BC), np.float32),
        onesb=_bf16(np.ones((1, TT))), pew=g['PEW'], phw=g['PHW'], gw=g['GW'],
        fbias=g['FBIAS'], pet=g['PET'],
        ones=np.ones((128, 128), np.float32),
    )
    Hp = 224 // P
    pat = x.reshape(B, 3, Hp, P, Hp, P).transpose(0, 1, 2, 4, 3, 5).reshape(B, 3, NP_, 2, 128)
    maps = []
    for i in range(NCORES):
        m = dict(shared)
        pc = pat[i * BC:(i + 1) * BC]                       # [BC,3,196,2,128]
        m['patt'] = np.ascontiguousarray(pc.transpose(4, 1, 3, 0, 2).reshape(128, 3, 2, NBP))
        m['pft'] = np.ascontiguousarray(pf[i * BC:(i + 1) * BC].reshape(NBP, 6).T)
        maps.append(m)
    return maps


def kernel(**inputs):
    inputs = {k: np.asarray(v) for k, v in inputs.items()}
    g = _prep(inputs)
    # program structure bakes per-layer ln1 gains into immediates; key on them
    key = (tuple(np.round(np.asarray(g['cg'], np.float64), 12)),)
    if _CACHE.get('key') != key:
        _CACHE['prog'] = _build(g)
        _CACHE['key'] = key
    nc = _CACHE['prog']
    res = run_bass_kernel_spmd(nc, _in_maps(inputs, g), list(range(NCORES)))
    _CACHE['last_res'] = res
    _CACHE['last_g'] = g
    hcls = np.concatenate(
        [r['hcls'].transpose(1, 0, 2).reshape(D, BC) for r in res.results], axis=1)
    return _head(hcls, g)


if __name__ == '__main__':
    d = np.load('/root/problem/ref_data.npz')
    inputs = {k: d[k] for k in d.files if k != 'expected'}
    y = kernel(**inputs)
    exp = d['expected']
    err = np.abs(y - exp)
    print("max abs err:", err.max())
    print("Relative error:", err.max() / np.abs(exp).max())


# revision 52
# speedup vs baseline: 1.0862x; 1.0009x over previous
"""Trainium2 Bass kernel for nn_FFTPermeabilityPredictorPatchPhysics.

Sharding: pure data parallel — 8 samples per NeuronCore, weights replicated.
On-device layout: residual stream transposed, hT [3x128 d-chunks, 1576 tok],
kept in SBUF for all 12 layers. FFT/iFFT as block-diagonal matmuls over a
512-row padded frequency layout (head h -> rows 64h+32s+f). LN stats via
ones-matmul partition reductions broadcast to all partitions; the adaptive
spectral filter is fused into the ACT-engine gelu via per-partition
scale/bias. The MLP runs fp8e4 DoubleRow (K=256 per instruction) for the
196 patch tokens with weight scale 64 folded into the gelu scale and the
residual scalar_tensor_tensor; the cls token column (which feeds the head
directly, without the 1/197 mean dilution of patch tokens) is recomputed
in bf16 against the same-layer bf16 weights. All weight folding done
host-side in numpy: double-LN collapse, pre_g/ln2_g into following
matmuls, base_filter and (1+ap) into amlp_w2, 1/197 token-mean into
amlp_w1, DFT matrices baked. Final LN + head on the 64 cls vectors runs
host-side in float64.
"""
import numpy as np

import concourse.bacc as bacc
import concourse.mybir as mybir
import concourse.tile as tile
from concourse.bass_utils import run_bass_kernel_spmd

F32 = mybir.dt.float32
F32R = mybir.dt.float32r
BF16 = mybir.dt.bfloat16
FP8 = mybir.dt.float8e4
AF = mybir.ActivationFunctionType
ALU = mybir.AluOpType
DR = mybir.MatmulPerfMode.DoubleRow

B, D, H, HD, FB, S, L, P, NP_ = 64, 384, 8, 48, 25, 197, 12, 16, 196
EPS = 1e-5
FR = 512
NCORES = 8
BC = B // NCORES     # 8 samples/core
NTOK = BC * S        # 1576
TT = 394             # token tile = 2 samples
NBP = BC * NP_       # 1568
BT = 392             # patch tile = 2 samples
WS = 64.0            # fp8 weight scale for both MLP matmuls
IWS = 1.0 / WS

_CACHE = {}


def _build_dft():
    n = np.arange(HD)
    k = np.arange(FB)
    ang = -2 * np.pi * np.outer(n, k) / HD
    Cr = np.cos(ang) / np.sqrt(HD)
    Ci = np.sin(ang) / np.sqrt(HD)
    A = np.zeros((FB, HD))
    Bm = np.zeros((FB, HD))
    ifft_w = np.exp(2j * np.pi * np.outer(np.arange(HD), np.arange(HD)) / HD) / np.sqrt(HD)
    for j in range(FB):
        fr = np.zeros(HD, complex)
        fi = np.zeros(HD, complex)
        fr[j] = 1.0
        fi[j] = 1.0j
        if 0 < j < HD - FB + 1:
            fr[HD - j] = 1.0
            fi[HD - j] = -1.0j
        A[j] = (ifft_w @ fr).real
        Bm[j] = (ifft_w @ fi).real
    return Cr, Ci, A, Bm


def _prep(inp, n_layers=L):
    f = {k: np.asarray(v, np.float64) for k, v in inp.items()}
    Cr, Ci, A, Bm = _build_dft()

    BDb = np.zeros((D, FR))
    iBD = np.zeros((FR, D))
    for h in range(H):
        BDb[48 * h:48 * h + 48, 64 * h:64 * h + FB] = Cr
        BDb[48 * h:48 * h + 48, 64 * h + 32:64 * h + 32 + FB] = Ci
        iBD[64 * h:64 * h + FB, 48 * h:48 * h + 48] = A
        iBD[64 * h + 32:64 * h + 32 + FB, 48 * h:48 * h + 48] = Bm

    cg = f['ln1_g'].mean(1)
    assert np.abs(f['ln1_g'] - cg[:, None]).max() < 1e-12, "ln1_g must be constant/layer"
    assert np.abs(f['ln1_b'] - f['ln1_b'].mean(1)[:, None]).max() < 1e-12
    assert np.allclose(f['pe_ln_g'], 1.0) and np.allclose(f['pe_ln_b'], 0.0), "pe_ln fold"

    BD_l = np.einsum('ld,df->ldf', cg[:, None] * f['pre_g'], BDb)
    bdbias_l = np.einsum('ld,df->lf', f['pre_b'], BDb)

    aw1p = np.einsum('ld,lde->lde', cg[:, None] * f['pre_g'], f['amlp_w1']) / S
    ab1p = np.einsum('ld,lde->le', f['pre_b'], f['amlp_w1']) + f['amlp_b1']

    aw2pp = np.zeros((L, D, 2 * FR))
    ab2pp = np.zeros((L, 2 * FR))
    aw2, ab2 = f['amlp_w2'], f['amlp_b2']
    bf, bb = f['base_filter'], f['base_bias']
    for h in range(H):
        for s in range(2):
            for fq in range(FB):
                r = 64 * h + 32 * s + fq
                c0 = h * (FB * 2) + fq * 2
                wf = bf[:, h, fq][:, None] * aw2[:, :, c0]
                bf_ = bf[:, h, fq] * ab2[:, c0] + bf[:, h, fq]
                aw2pp[:, :, r] = wf
                ab2pp[:, r] = bf_
                aw2pp[:, :, FR + r] = bdbias_l[:, r][:, None] * wf
                ab2pp[:, FR + r] = bdbias_l[:, r] * bf_
                if s == 0:
                    aw2pp[:, :, FR + r] += aw2[:, :, c0 + 1]
                    ab2pp[:, FR + r] += bb[:, h, fq] + ab2[:, c0 + 1]

    w1p = np.einsum('ld,lde->lde', f['ln2_g'], f['mlp_w1'])
    b1p = np.einsum('ld,lde->le', f['ln2_b'], f['mlp_w1']) + f['mlp_b1']

    a32 = lambda x: np.ascontiguousarray(x, np.float32)
    g = {}
    g['cg'] = cg
    g['W1'] = _bf16(w1p.reshape(L, 3, 128, 4 * D).transpose(0, 2, 1, 3))            # [L,128,3,1536] bf16
    g['W2'] = _bf16(f['mlp_w2'].reshape(L, 12, 128, 3, 128).transpose(0, 2, 1, 3, 4))
    # fp8 copies (scaled by WS); W1 padded to 4 k-chunks for DoubleRow pairs
    w18 = np.zeros((L, 128, 4, 4 * D))
    w18[:, :, :3, :] = WS * w1p.reshape(L, 3, 128, 4 * D).transpose(0, 2, 1, 3)
    g['W18'] = _fp8(w18)                                                            # [L,128,4,1536]
    g['W28'] = _fp8(WS * f['mlp_w2'].reshape(L, 12, 128, 3, 128).transpose(0, 2, 1, 3, 4))
    g['BD'] = a32(BD_l.reshape(L, 3, 128, 4, 128).transpose(0, 2, 1, 3, 4))
    g['IBD'] = a32(iBD.reshape(4, 128, 3, 128).transpose(1, 0, 2, 3))
    g['AW1'] = a32(aw1p.reshape(L, 3, 128, D).transpose(0, 2, 1, 3))
    g['AB2R'] = a32(ab2pp[:, None, :])                                              # [L,1,1024]
    g['B2R'] = _bf16(f['mlp_b2'][:, None, :].reshape(L, 1, 3, 128))
    g['B2R8'] = _bf16(WS * f['mlp_b2'][:, None, :].reshape(L, 1, 3, 128))
    g['B1R'] = _bf16(b1p[:, None, :].reshape(L, 1, 12, 128))
    g['AW2'] = a32(aw2pp.reshape(L, 3, 128, 2 * FR).transpose(0, 2, 1, 3))
    # packed per-layer biases [L,128,26]: 0-2 ab1, 3-10 ab2, 11-22 b1, 23-25 b2
    bias = np.zeros((L, 128, 26))
    bias[:, :, 0:3] = ab1p.reshape(L, 3, 128).transpose(0, 2, 1)
    bias[:, :, 3:11] = ab2pp.reshape(L, 8, 128).transpose(0, 2, 1)
    bias[:, :, 11:23] = b1p.reshape(L, 12, 128).transpose(0, 2, 1)
    bias[:, :, 23:26] = f['mlp_b2'].reshape(L, 3, 128).transpose(0, 2, 1)
    g['BIAS'] = a32(bias)
    g['PEW'] = a32(f['pe_w'].reshape(3, 2, 128, 128).transpose(2, 0, 1, 3))          # [128,3,2,128]
    g['PHW'] = a32(f['phys_w'].reshape(6, 3, 128))                                   # [6,3,128]
    g['GW'] = a32(f['gate_w'].reshape(6, 128, 3, 128).transpose(1, 0, 2, 3))         # [128,6,3,128]
    fbias = np.zeros((128, 12))  # 0-2 peb, 3-5 phb, 6-8 gb, 9-11 clspe
    fbias[:, 0:3] = f['pe_b'].T
    fbias[:, 3:6] = f['phys_b'].reshape(3, 128).T
    fbias[:, 6:9] = f['gate_b'].reshape(3, 128).T
    fbias[:, 9:12] = (f['cls_token'][0, 0] + f['pos_embed'][0, 0]).reshape(3, 128).T
    g['FBIAS'] = a32(fbias)
    g['PET'] = a32(f['pos_embed'][0, 1:].T.reshape(3, 128, NP_).transpose(1, 0, 2))  # [128,3,196]
    for kk in ('norm_g', 'norm_b', 'head_w1', 'head_b1', 'head_w2', 'head_b2'):
        g[kk] = f[kk]
    g['n_layers'] = n_layers
    return g


def _bf16(x):
    import ml_dtypes
    return np.ascontiguousarray(np.asarray(x, np.float32), dtype=ml_dtypes.bfloat16)


def _fp8(x):
    import ml_dtypes
    return np.ascontiguousarray(np.asarray(x, np.float32), dtype=ml_dtypes.float8_e4m3)


def _build(g):
    import math
    n_layers = g['n_layers']
    nc = bacc.Bacc('TRN2', target_bir_lowering=False, debug=False)
    for val in (EPS,):
        t = nc.alloc_sbuf_tensor(f"const-f32-{val}", [128, 1], F32)
        nc.gpsimd.memset(t.ap(), val)
        nc.const_aps.aps[(F32, val)] = t.ap()
    nc.all_engine_barrier()

    di = lambda name, shape, dt: nc.dram_tensor(name, list(shape), dt, kind="ExternalInput")
    PATd = di('patt', (128, 3, 2, NBP), F32R)
    PFT = di('pft', (6, NBP), F32R)
    W1d = di('w1', (L, 128, 3, 1536), BF16)
    W2d = di('w2', (L, 128, 12, 3, 128), BF16)
    W18d = di('w18', (L, 128, 4, 1536), FP8)
    W28d = di('w28', (L, 128, 12, 3, 128), FP8)
    BDd = di('bd', (L, 128, 3, 4, 128), F32R)
    IBDd = di('ibd', (128, 4, 3, 128), F32R)
    AW1d = di('aw1', (L, 128, 3, 384), F32)
    AW2d = di('aw2', (L, 128, 3, 1024), F32)
    BIASd = di('bias', (L, 128, 26), F32)
    AB2Rd = di('ab2r', (L, 1, 1024), F32)
    B2Rd = di('b2r', (L, 1, 3, 128), BF16)
    B2R8d = di('b2r8', (L, 1, 3, 128), BF16)
    B1Rd = di('b1r', (L, 1, 12, 128), BF16)
    ONFd = di('onesf', (1, BC), F32)
    ONBd = di('onesb', (1, TT), BF16)
    PEWd = di('pew', (128, 3, 2, 128), F32R)
    PHWd = di('phw', (6, 3, 128), F32R)
    GWd = di('gw', (128, 6, 3, 128), F32R)
    FBIASd = di('fbias', (128, 12), F32)
    PETd = di('pet', (128, 3, NP_), F32)
    ONESd = di('ones', (128, 128), F32R)
    HCLS = nc.dram_tensor('hcls', [128, 3, BC], F32, kind="ExternalOutput")

    with tile.TileContext(nc) as tc:
        with (
            tc.tile_pool(name='const', bufs=1) as cp,
            tc.tile_pool(name='persist', bufs=1) as pp,
            tc.tile_pool(name='hnp', bufs=1) as hnp,
            tc.tile_pool(name='xqp', bufs=4) as xqp,
            tc.tile_pool(name='stp', bufs=4) as stp,
            tc.tile_pool(name='psp', bufs=6, space='PSUM') as psp,
        ):
            ones_t = cp.tile([128, 128], F32R, name='ones_t')
            nc.sync.dma_start(ones_t[:], ONESd[:])
            ibd_t = cp.tile([128, 4, 3, 128], F32R, name='ibd_t')
            nc.sync.dma_start(ibd_t[:], IBDd[:])
            onesf_t = cp.tile([1, BC], F32, name='onesf_t')
            nc.sync.dma_start(onesf_t[:], ONFd[:])
            onesb_t = cp.tile([1, TT], BF16, name='onesb_t')
            nc.sync.dma_start(onesb_t[:], ONBd[:])
            fbias_t = cp.tile([128, 12], F32, name='fbias_t')
            nc.sync.dma_start(fbias_t[:], FBIASd[:])
            pet_t = cp.tile([128, 3, NP_], F32, name='pet_t')
            nc.sync.dma_start(pet_t[:], PETd[:])

            hT = pp.tile([128, 3, NTOK], F32, name='hT')

            def stats_pre(srcs, tlen, cgl=None, pstag='ps', on_act=False):
                """LN stats (up to 1/ve) for one token tile; srcs = 3
                [128,tlen] f32 APs. Double-LN folds to a single rsqrt:
                rs1*rs2 = rsqrt((cg^2+eps)*v + eps^2). Act-table-free:
                the Sqrt is emitted separately by stats_sqrt."""
                xq = xqp.tile([128, 3, TT], F32R, tag='xq', name='xq')
                for c in range(3):
                    eng = nc.vector if c == 0 else nc.gpsimd
                    eng.tensor_mul(xq[:, c, :tlen], srcs[c], srcs[c])
                ps_s = psp.tile([128, TT], F32, tag='ps2', bufs=2, name='ps_s')
                ps_q = psp.tile([128, TT], F32, tag='ps2', bufs=2, name='ps_q')
                for c in range(3):
                    nc.tensor.matmul(ps_s[:, :tlen], ones_t[:], srcs[c].bitcast(F32R),
                                     start=(c == 0), stop=(c == 2))
                for c in range(3):
                    nc.tensor.matmul(ps_q[:, :tlen], ones_t[:], xq[:, c, :tlen],
                                     start=(c == 0), stop=(c == 2))
                if cgl is None:
                    A, Bc_ = 1.0, EPS
                else:
                    A = float(cgl) * float(cgl) + EPS
                    Bc_ = EPS * EPS
                st = stp.tile([128, 5, TT], F32, tag='st', name='st')
                m = st[:, 0, :tlen]
                rsd = st[:, 1, :tlen]
                mm = st[:, 2, :tlen]
                t1 = st[:, 3, :tlen]
                ve = st[:, 4, :tlen]
                u = st[:, 3, :tlen]  # t1's row; t1 is dead once ve is formed
                # PSUM readers must be DVE/Act (GPSIMD cannot access PSUM).
                # on_act runs the PSUM-consuming stats on the Act engine
                # (Copy/Square live in every act table): used for the
                # next-layer stats at the tail of a layer, where Act idles
                # and early PSUM reads unblock the next layer's matmuls.
                if on_act:
                    nc.scalar.activation(m, ps_s[:, :tlen], AF.Copy, scale=1.0 / D)
                    nc.scalar.activation(t1, ps_q[:, :tlen], AF.Copy, bias=Bc_,
                                         scale=A / D)
                    nc.scalar.activation(mm, ps_s[:, :tlen], AF.Square,
                                         scale=math.sqrt(A) / D)
                    nc.vector.scalar_tensor_tensor(ve, mm, -1.0, t1, ALU.mult, ALU.add)
                else:
                    # B (eps^2 for the folded double-LN, eps for LN2) is
                    # negligible vs ve ~ A*var = O(1): drop it and save an op.
                    nc.vector.tensor_scalar(m, ps_s[:, :tlen], 1.0 / D, None, ALU.mult)
                    nc.vector.scalar_tensor_tensor(mm, m, A, m, ALU.mult, ALU.mult)
                    nc.vector.scalar_tensor_tensor(ve, ps_q[:, :tlen], A / D, mm,
                                                   ALU.mult, ALU.subtract)
                nc.vector.reciprocal(u, ve)
                return st

            def stats_sqrt(st, tlen, gate=None):
                if gate is None:
                    nc.scalar.activation(st[:, 1, :tlen], st[:, 3, :tlen], AF.Sqrt)
                else:
                    nc.scalar.activation(st[:, 1, :tlen], st[:, 3, :tlen], AF.Sqrt,
                                         scale=gate)

            def sqrt_gang(sts_list, tlen):
                """Emit the sqrts of a stats batch gated on the LAST tile's
                recip, so the Act-engine scheduler runs them back-to-back
                (one sqrt<->gelu table swap per batch instead of one per
                tile)."""
                gate = stp.tile([128, 1], F32, tag='gate', name='gate', bufs=4)
                last_u = sts_list[-1][:, 3, 0:1]
                nc.vector.tensor_scalar(gate, last_u, 0.0, 1.0, ALU.mult, ALU.add)
                for st in sts_list:
                    stats_sqrt(st, tlen, gate=gate)

            def mlp_tile8(sl, h2, w18_t, w28_t, b2r8_t, onesb_t, bias_t):
                """fp8 DoubleRow MLP for one 2-sample token tile; the two cls
                columns of the residual are left to the bf16 cls path."""
                mid = midp.tile([128, 12, TT], FP8, tag='mid', name='mid')
                for grp in range(3):
                    pss = []
                    for mci in range(4):
                        mc = grp * 4 + mci
                        ps_m = psp.tile([128, TT], F32, tag='ps', name='ps_m')
                        for j in range(2):
                            nc.tensor.matmul(
                                ps_m[:], w18_t[:, 2 * j:2 * j + 2, mc * 128:(mc + 1) * 128],
                                h2[:, 2 * j:2 * j + 2, :], start=(j == 0), stop=(j == 1),
                                perf_mode=DR)
                        pss.append((mc, ps_m))
                    for mc, ps_m in pss:
                        nc.scalar.activation(mid[:, mc, :], ps_m[:], AF.Gelu,
                                             scale=IWS, bias=bias_t[:, 11 + mc:12 + mc])
                for mc in range(3):
                    ps_o = psp.tile([128, TT], F32, tag='ps', name='ps_o')
                    for j in range(6):
                        nc.tensor.matmul(ps_o[:], w28_t[:, 2 * j:2 * j + 2, mc, :],
                                         mid[:, 2 * j:2 * j + 2, :],
                                         start=(j == 0), stop=False, perf_mode=DR)
                    nc.tensor.matmul(ps_o[:], b2r8_t[:, mc, :], onesb_t[0:1, :TT],
                                     start=False, stop=True)
                    nc.vector.scalar_tensor_tensor(
                        hT[:, mc, sl].bitcast(F32R), ps_o[:], IWS,
                        hT[:, mc, sl], ALU.mult, ALU.add)
                    pcls = ps_o.rearrange("p (j s) -> p j s", s=S)[:, :, 0]
                    htc = hT[:, mc, sl].rearrange("p (j s) -> p j s", s=S)[:, :, 0]
                    nc.vector.scalar_tensor_tensor(
                        htc.bitcast(F32R), pcls, -IWS, htc, ALU.mult, ALU.add)

            # ================= front (streamed per 2-sample group) ==========
            with (
                tc.tile_pool(name='fgrp', bufs=3) as fg_,
                tc.tile_pool(name='fw', bufs=1) as fw,
            ):
                pft_t = fw.tile([6, NBP], F32R, name='pft_t')
                nc.sync.dma_start(pft_t[:], PFT[:])
                pew_t = fw.tile([128, 3, 2, 128], F32R, name='pew_t')
                nc.sync.dma_start(pew_t[:], PEWd[:])
                phw_t = fw.tile([6, 3, 128], F32R, name='phw_t')
                nc.sync.dma_start(phw_t[:], PHWd[:])
                for grp in range(4):
                    sl = slice(grp * BT, (grp + 1) * BT)
                    patg = fg_.tile([128, 3, 2, BT], F32R, tag='patg', name='patg')
                    for c in range(3):
                        nc.sync.dma_start(patg[:, c], PATd[:, c, :, sl])
                    ximg = fg_.tile([128, 3, BT], F32R, tag='ximg', name='ximg')
                    xn = fg_.tile([128, 3, BT], F32R, tag='xn', name='xn')
                    xp = fg_.tile([128, 3, BT], F32R, tag='xp', name='xp')
                    gt = fg_.tile([128, 3, BT], F32, tag='gt', name='gt')
                    for c in range(3):
                        ps_pe = psp.tile([128, TT], F32, tag='ps', name='ps_pe')
                        for kc in range(2):
                            nc.tensor.matmul(ps_pe[:, :BT], pew_t[:, c, kc, :], patg[:, c, kc, :],
                                             start=(kc == 0), stop=(kc == 1))
                        nc.scalar.activation(ximg[:, c, :], ps_pe[:, :BT], AF.Identity,
                                             bias=fbias_t[:, c:c + 1])
                    if grp == 0:
                        gw_t = fw.tile([128, 6, 3, 128], F32R, name='gw_t')
                        nc.sync.dma_start(gw_t[:], GWd[:])
                    xi = [ximg[:, c, :].bitcast(F32) for c in range(3)]
                    st = stats_pre(xi, BT)
                    stats_sqrt(st, BT)
                    m = st[:, 0, :BT]
                    rsd = st[:, 1, :BT]
                    for c in range(3):
                        eng = nc.gpsimd if c == 2 else nc.vector
                        tm = st[:, 2 + c, :BT]
                        eng.tensor_sub(tm, xi[c], m)
                        eng.tensor_mul(xn[:, c, :], tm, rsd)
                    for mc in range(3):
                        ps_ph = psp.tile([128, TT], F32, tag='ps', name='ps_ph')
                        nc.tensor.matmul(ps_ph[:, :BT], phw_t[:, mc, :], pft_t[:, sl],
                                         start=True, stop=True)
                        nc.scalar.activation(xp[:, mc, :], ps_ph[:, :BT], AF.Identity,
                                             bias=fbias_t[:, 3 + mc:4 + mc])
                    for mc in range(3):
                        ps_g = psp.tile([128, TT], F32, tag='ps', name='ps_g')
                        for kc in range(6):
                            rhs = xn[:, kc, :] if kc < 3 else xp[:, kc - 3, :]
                            nc.tensor.matmul(ps_g[:, :BT], gw_t[:, kc, mc, :], rhs,
                                             start=(kc == 0), stop=(kc == 5))
                        nc.scalar.activation(gt[:, mc, :], ps_g[:, :BT], AF.Sigmoid,
                                             bias=fbias_t[:, 6 + mc:7 + mc])
                    for bl in range(2):
                        b = 2 * grp + bl
                        psl = slice(bl * NP_, (bl + 1) * NP_)
                        tsl = slice(b * S + 1, (b + 1) * S)
                        dd = stp.tile([128, 5, TT], F32, tag='st', name='fd')
                        dv = dd[:, 0:3, :NP_]
                        nc.vector.tensor_sub(dv, xn[:, :, psl].bitcast(F32), xp[:, :, psl].bitcast(F32))
                        nc.vector.tensor_mul(dv, gt[:, :, psl], dv)
                        nc.vector.tensor_add(dv, dv, xp[:, :, psl].bitcast(F32))
                        nc.vector.tensor_add(hT[:, :, tsl].bitcast(F32R), dv, pet_t[:])
                        nc.vector.tensor_copy(hT[:, :, b * S:b * S + 1].bitcast(F32R),
                                              fbias_t[:, 9:12].unsqueeze(2))

            # ========================= transformer layers ===================
            with (
                tc.tile_pool(name='w1bp', bufs=1) as w1bp,
                tc.tile_pool(name='w2bp', bufs=1) as w2bp,
                tc.tile_pool(name='w18p', bufs=2) as w18p,
                tc.tile_pool(name='w28p', bufs=2) as w28p,
                tc.tile_pool(name='wps', bufs=1) as wps,
                tc.tile_pool(name='fgp', bufs=2) as fgp,
                tc.tile_pool(name='midp', bufs=2) as midp,
                tc.tile_pool(name='h2p', bufs=4) as h2p,
                tc.tile_pool(name='clsp', bufs=2) as clsp,
                tc.tile_pool(name='amp', bufs=1) as amp,
            ):
                pending = None
                for l in range(n_layers):
                    w1_t = w1bp.tile([128, 3, 1536], BF16, tag='w1b', name='w1_t')
                    nc.sync.dma_start(w1_t[:], W1d[l])
                    w2_t = w2bp.tile([128, 12, 3, 128], BF16, tag='w2b', name='w2_t')
                    nc.sync.dma_start(w2_t[:], W2d[l])
                    w18_t = w18p.tile([128, 4, 1536], FP8, tag='w18', name='w18_t')
                    nc.sync.dma_start(w18_t[:], W18d[l])
                    w28_t = w28p.tile([128, 12, 3, 128], FP8, tag='w28', name='w28_t')
                    nc.sync.dma_start(w28_t[:], W28d[l])
                    bd_t = wps.tile([128, 3, 4, 128], F32R, tag='bd', name='bd_t')
                    nc.sync.dma_start(bd_t[:], BDd[l])
                    aw1_t = wps.tile([128, 3, 384], F32, tag='aw1', name='aw1_t')
                    nc.sync.dma_start(aw1_t[:], AW1d[l])
                    aw2_t = wps.tile([128, 3, 1024], F32, tag='aw2', name='aw2_t')
                    nc.sync.dma_start(aw2_t[:], AW2d[l])
                    bias_t = wps.tile([128, 26], F32, tag='bias', name='bias_t')
                    nc.sync.dma_start(bias_t[:], BIASd[l])
                    ab2r_t = wps.tile([1, 1024], F32, tag='ab2r', name='ab2r_t')
                    nc.sync.dma_start(ab2r_t[:], AB2Rd[l])
                    b2r_t = wps.tile([1, 3, 128], BF16, tag='b2r', name='b2r_t')
                    nc.sync.dma_start(b2r_t[:], B2Rd[l])
                    b2r8_t = wps.tile([1, 3, 128], BF16, tag='b2r8', name='b2r8_t')
                    nc.sync.dma_start(b2r8_t[:], B2R8d[l])
                    b1r_t = wps.tile([1, 12, 128], BF16, tag='b1r', name='b1r_t')
                    nc.sync.dma_start(b1r_t[:], B1Rd[l])

                    hn = hnp.tile([128, 3, NTOK], F32R, tag='hn', name='hn')
                    mh = amp.tile([128, 3, BC], F32, tag='mh', name='mh')
                    if pending is None:
                        sts = []
                        for t in range(4):
                            sl = slice(t * TT, (t + 1) * TT)
                            hs = [hT[:, c, sl] for c in range(3)]
                            sts.append((sl, hs, stats_pre(hs, TT, cgl=g['cg'][l])))
                        sqrt_gang([sts[t][2] for t in range(4)], TT)
                    else:
                        sts = pending
                    ps_u = psp.tile([128, TT], F32, tag='ps2', bufs=2, name='ps_u')
                    ps_e = psp.tile([128, TT], F32, tag='ps2', bufs=2, name='ps_e')
                    u2t = amp.tile([128, 3, BC], F32, tag='u2', name='u2t')
                    eff = amp.tile([128, 8, BC], F32, tag='eff', name='eff')
                    for t in range(4):
                        sl, hs, st = sts[t]
                        m = st[:, 0, :]
                        rsd = st[:, 1, :]
                        # LN1 apply with fused per-sample token-sum (-> mh)
                        for c in range(3):
                            tm = st[:, 2 + c, :]
                            eng = nc.gpsimd if c == 2 else nc.vector
                            eng.tensor_sub(tm, hs[c], m)
                            for j in range(2):
                                jsl = slice(j * S, (j + 1) * S)
                                nc.vector.scalar_tensor_tensor(
                                    hn[:, c, sl][:, jsl], tm[:, jsl], 1.0,
                                    rsd[:, jsl], ALU.mult, ALU.mult,
                                    accum_out=mh[:, c, 2 * t + j:2 * t + j + 1])
                        bsl = slice(2 * t, 2 * t + 2)
                        for mc in range(3):
                            for kc in range(3):
                                nc.tensor.matmul(
                                    ps_u[:, mc * BC:mc * BC + BC][:, bsl],
                                    aw1_t[:, kc, mc * 128:(mc + 1) * 128],
                                    mh[:, kc, bsl], start=(kc == 0), stop=(kc == 2))
                        if t in (1, 3):
                            hsl = slice(0, 4) if t == 1 else slice(4, 8)
                            for mc in range(3):
                                nc.scalar.activation(u2t[:, mc, hsl],
                                                     ps_u[:, mc * BC:mc * BC + BC][:, hsl],
                                                     AF.Gelu, bias=bias_t[:, mc:mc + 1])
                            for mt in range(8):
                                for kc in range(3):
                                    nc.tensor.matmul(
                                        ps_e[:, mt * BC:mt * BC + BC][:, hsl],
                                        aw2_t[:, kc, mt * 128:(mt + 1) * 128],
                                        u2t[:, kc, hsl], start=(kc == 0), stop=False)
                                nc.tensor.matmul(
                                    ps_e[:, mt * BC:mt * BC + BC][:, hsl],
                                    ab2r_t[:, mt * 128:(mt + 1) * 128],
                                    onesf_t[0:1, hsl], start=False, stop=True)
                                nc.vector.tensor_scalar(eff[:, mt, hsl],
                                                        ps_e[:, mt * BC:mt * BC + BC][:, hsl],
                                                        1.0, None, ALU.mult)  # PSUM read: DVE

                    # FFT mixer
                    KCS_F = [[0], [0, 1], [1, 2], [2]]
                    KCS_I = [[0, 1], [1, 2], [2, 3]]
                    for t in range(4):
                        sl = slice(t * TT, (t + 1) * TT)
                        fg = fgp.tile([128, 4, TT], F32R, tag='fg', name='fg')
                        for mc in range(4):
                            ps_F = psp.tile([128, TT], F32, tag='ps', name='ps_F')
                            kcs = KCS_F[mc]
                            for i, kc in enumerate(kcs):
                                nc.tensor.matmul(ps_F[:], bd_t[:, kc, mc, :], hn[:, kc, sl],
                                                 start=(i == 0), stop=(i == len(kcs) - 1))
                            for j in range(2):
                                bb = 2 * t + j
                                nc.scalar.activation(fg[:, mc, j * S:(j + 1) * S],
                                                     ps_F[:, j * S:(j + 1) * S], AF.Gelu,
                                                     scale=eff[:, mc, bb:bb + 1],
                                                     bias=eff[:, 4 + mc, bb:bb + 1])
                        for mc in range(3):
                            ps_A = psp.tile([128, TT], F32, tag='ps', name='ps_A')
                            kcs = KCS_I[mc]
                            for i, kc in enumerate(kcs):
                                nc.tensor.matmul(ps_A[:], ibd_t[:, kc, mc, :], fg[:, kc, :],
                                                 start=(i == 0), stop=(i == len(kcs) - 1))
                            nc.vector.tensor_add(hT[:, mc, sl].bitcast(F32R), hT[:, mc, sl], ps_A[:])

                    # LN2 stats (sqrt batched after all 4 tiles)
                    ln2q = []
                    for t in range(4):
                        sl = slice(t * TT, (t + 1) * TT)
                        hs = [hT[:, c, sl] for c in range(3)]
                        ln2q.append((sl, hs, stats_pre(hs, TT)))
                        if t in (1, 3):
                            sqrt_gang([ln2q[t - 1][2], ln2q[t][2]], TT)

                    # LN2 apply -> h2 fp8 (+ bf16 cls columns)
                    h2cls = clsp.tile([128, 3, BC], BF16, tag='h2c', name='h2cls')
                    h2s = []
                    for t in range(4):
                        sl, hs, st = ln2q[t]
                        m = st[:, 0, :]
                        rsd = st[:, 1, :]
                        h2 = h2p.tile([128, 4, TT], FP8, tag='h2', name='h2')
                        nc.gpsimd.memset(h2[:, 3, :], 0.0)
                        for c in range(3):
                            tm = st[:, 2 + c, :]
                            eng = nc.gpsimd if c == 2 else nc.vector
                            eng.tensor_sub(tm, hs[c], m)
                            nc.gpsimd.tensor_mul(h2[:, c, :], tm, rsd)
                            tm2 = tm.rearrange("p (j s) -> p j s", s=S)[:, :, 0]
                            rs2 = rsd.rearrange("p (j s) -> p j s", s=S)[:, :, 0]
                            nc.vector.scalar_tensor_tensor(h2cls[:, c, 2 * t:2 * t + 2],
                                                           tm2, 1.0, rs2, ALU.mult, ALU.mult)
                        h2s.append(h2)

                    # cls-column MLP in bf16 (8 columns, all samples at once)
                    midcls = clsp.tile([128, 12, BC], BF16, tag='midc', name='midcls')
                    ps_c1 = psp.tile([128, 12 * BC], F32, tag='ps', name='ps_c1')
                    for mc in range(12):
                        oc = ps_c1[:, mc * BC:(mc + 1) * BC]
                        for kc in range(3):
                            nc.tensor.matmul(oc, w1_t[:, kc, mc * 128:(mc + 1) * 128],
                                             h2cls[:, kc, :], start=(kc == 0), stop=False)
                        nc.tensor.matmul(oc, b1r_t[:, mc, :], onesb_t[0:1, :BC],
                                         start=False, stop=True)
                    nc.scalar.activation(midcls[:].rearrange("p a b -> p (a b)"), ps_c1[:],
                                         AF.Gelu)
                    ps_c2 = psp.tile([128, 3 * BC], F32, tag='ps', name='ps_c2')
                    for mc in range(3):
                        oc = ps_c2[:, mc * BC:(mc + 1) * BC]
                        for kc in range(12):
                            nc.tensor.matmul(oc, w2_t[:, kc, mc, :], midcls[:, kc, :],
                                             start=(kc == 0), stop=False)
                        nc.tensor.matmul(oc, b2r_t[:, mc, :], onesb_t[0:1, :BC],
                                         start=False, stop=True)
                    for mc in range(3):
                        ht_cls = hT[:, mc, :].rearrange("p (b s) -> p b s", s=S)[:, :, 0]
                        nc.vector.scalar_tensor_tensor(ht_cls.bitcast(F32R), ps_c2[:, mc * BC:(mc + 1) * BC],
                                                       1.0, ht_cls, ALU.mult, ALU.add)

                    # fp8 MLP (patch tokens; cls columns of the residual are
                    # skipped — the bf16 path above owns them)
                    nxt = []
                    for t in range(4):
                        mlp_tile8(ln2q[t][0], h2s[t], w18_t, w28_t, b2r8_t,
                                  onesb_t, bias_t)
                        sl, hs, _ = ln2q[t]
                        if l + 1 < n_layers:
                            nxt.append((sl, hs, stats_pre(hs, TT, cgl=g['cg'][l + 1])))
                            if t in (1, 3):
                                sqrt_gang([nxt[t - 1][2], nxt[t][2]], TT)
                    pending = nxt if l + 1 < n_layers else None

                out_ap = hT[:].rearrange("p c (b s) -> p c b s", s=S)[:, :, :, 0]
                nc.sync.dma_start(HCLS[:], out_ap)

    nc.compile()
    return nc


def _gelu_np(x):
    try:
        from scipy.special import erf
    except ImportError:
        import math
        erf = np.vectorize(math.erf)
    return x * 0.5 * (1.0 + erf(x / np.sqrt(2.0)))


def _head(hcls, g):
    x = hcls.astype(np.float64).T
    m = x.mean(1, keepdims=True)
    v = ((x - m) ** 2).mean(1, keepdims=True)
    cls = (x - m) / np.sqrt(v + EPS) * g['norm_g'] + g['norm_b']
    u = _gelu_np(cls @ g['head_w1'] + g['head_b1'])
    return ((u @ g['head_w2'])[:, 0] + g['head_b2'][0]).astype(np.float32)


def _in_maps(inputs, g):
    x = np.ascontiguousarray(inputs['x'], np.float32)
    pf = np.ascontiguousarray(inputs['patch_feats'], np.float32)
    shared = dict(
        w1=g['W1'], w2=g['W2'], w18=g['W18'], w28=g['W28'], bd=g['BD'],
        ibd=g['IBD'], aw1=g['AW1'],
        aw2=g['AW2'], bias=g['BIAS'], ab2r=g['AB2R'], b2r=g['B2R'],
        b2r8=g['B2R8'], b1r=g['B1R'],
        onesf=np.ones((1, BC), np.float32),
        onesb=_bf16(np.ones((1, TT))), pew=g['PEW'], phw=g['PHW'], gw=g['GW'],
        fbias=g['FBIAS'], pet=g['PET'],
        ones=np.ones((128, 128), np.float32),
    )
    Hp = 224 // P
    pat = x.reshape(B, 3, Hp, P, Hp, P).transpose(0, 1, 2, 4, 3, 5).reshape(B, 3, NP_, 2, 128)
    maps = []
    for i in range(NCORES):
        m = dict(shared)
        pc = pat[i * BC:(i + 1) * BC]                       # [BC,3,196,2,128]
        m['patt'] = np.ascontiguousarray(pc.transpose(4, 1, 3, 0, 2).reshape(128, 3, 2, NBP))
        m['pft'] = np.ascontiguousarray(pf[i * BC:(i + 1) * BC].reshape(NBP, 6).T)
        maps.append(m)
    return maps


def kernel(**inputs):
    inputs = {k: np.asarray(v) for k, v in inputs.items()}
    g = _prep(inputs)
    # program structure bakes per-layer ln1 gains into immediates; key on them
    key = (tuple(np.round(np.asarray(g['cg'], np.float64), 12)),)
    if _CACHE.get('key') != key:
        _CACHE['prog'] = _build(g)
        _CACHE['key'] = key
    nc = _CACHE['prog']
    res = run_bass_kernel_spmd(nc, _in_maps(inputs, g), list(range(NCORES)))
    _CACHE['last_res'] = res
    _CACHE['last_g'] = g
    hcls = np.concatenate(
        [r['hcls'].transpose(1, 0, 2).reshape(D, BC) for r in res.results], axis=1)
    return _head(hcls, g)


if __name__ == '__main__':
    d = np.load('/root/problem/ref_data.npz')
    inputs = {k: d[k] for k in d.files if k != 'expected'}
    y = kernel(**inputs)
    exp = d['expected']
    err = np.abs(y - exp)
    print("max abs err:", err.max())
    print("Relative error:", err.max() / np.abs(exp).max())
